# revision 24
# baseline (speedup 1.0000x reference)
"""Mamba block (dense_transformer nn_Block) on 8 Trainium2 NeuronCores.

Sharding: d_inner (2048 -> 256/core) for in_proj/conv/scan; per-batch-half
AllReduce for the small x_proj output; per-d-half AllToAll re-shards the scan
output to tokens (first half hidden under the second scan half); out_proj is a
split-K accumulation across the two A2A halves; MLP token-sharded. LayerNorms
fold into the following matmul (weights scaled host-side, mean correction via
a K=1 augmented matmul row, rstd applied in the PSUM epilogue).
"""
import os
import numpy as np
import ml_dtypes

import concourse.bass as bass
import concourse.bacc as bacc
import concourse.mybir as mybir
import concourse.tile as tile
from contextlib import ExitStack
from concourse.bass_utils import run_bass_kernel_spmd

BF16 = mybir.dt.bfloat16
F32 = mybir.dt.float32
AF = mybir.ActivationFunctionType
OP = mybir.AluOpType
bf = ml_dtypes.bfloat16

B, L, E = 2, 1024, 1024
DIN, NST, RDT, KC = 2 * E, 16, 64, 4
EPS = 1e-5
NC = 8
DL = DIN // NC          # 256 channels per core
TOK = B * L             # 2048
TOKC = TOK // NC        # 256 tokens per core post-A2A
HID = 4 * E             # 4096

_BUILD_CACHE = {}


def _rep0(src_ap, parts=128):
    """Partition-broadcast: prepend a [0, parts] dim to an AP's pattern."""
    return bass.AP(src_ap.tensor, src_ap.offset,
                   [[0, parts]] + [list(p) for p in src_ap.ap])


def _build(ln1b_nonzero):
    key = (ln1b_nonzero,)
    if key in _BUILD_CACHE:
        return _BUILD_CACHE[key]

    nc = bacc.Bacc("TRN2", target_bir_lowering=False, debug=False, num_devices=NC)

    def din(name, shape, dt=BF16):
        return nc.dram_tensor(name, shape, dt, kind="ExternalInput").ap()

    xT = din("xT", [E, TOK])
    win = din("win", [128, 8 * 512])
    sw_in = din("sw_in", [1, 512])
    sb_in = din("sb_in", [128, 4], F32)
    convw = din("convw", [128, 2 * KC], F32)
    convb = din("convb", [128, 2], F32)
    xpw = din("xpw", [128, 2 * 96])
    dtw = din("dtw", [64, 256])
    dtb = din("dtb", [128, 2], F32)
    a_sc = din("a_sc", [128, 2 * NST], F32)
    dvec = din("dvec", [128, 2], F32)
    wo = din("wo", [128, 16 * 1024])
    xres = din("xres", [E, TOKC], F32)
    wfc = din("wfc", [128, 8 * HID])
    swfc = din("swfc", [1, HID])
    sbfc = din("sbfc", [128, 32], F32)
    wpj = din("wpj", [128, 32 * E])
    pjb = din("pjb", [128, 8], F32)
    ones128 = din("ones128", [128, 1])
    ident = din("ident", [128, 128])

    outT = nc.dram_tensor("outT", [E, TOKC], F32, kind="ExternalOutput").ap()

    cc_dummy_in = nc.dram_tensor("cc_dummy_in", [1, 16], F32)
    cc_dummy_out = nc.dram_tensor("cc_dummy_out", [1, 16], F32, addr_space="Shared")
    ar_ins = [nc.dram_tensor(f"ar_in{b}", [96, L], F32) for b in range(2)]
    ar_outs = [nc.dram_tensor(f"ar_out{b}", [96, L], F32, addr_space="Shared")
               for b in range(2)]
    bc_bfs = [nc.dram_tensor(f"bc_bf{b}", [32, L], BF16) for b in range(2)]
    a2a_ins = [nc.dram_tensor(f"a2a_in{d}", [NC, 128 * TOKC], BF16)
               for d in range(2)]
    a2a_outs = [nc.dram_tensor(f"a2a_out{d}", [NC, 128 * TOKC], BF16)
                for d in range(2)]
    RG = [list(range(NC))]

    with tile.TileContext(nc) as tc, ExitStack() as _stk:
        # warm the collective stream early (absorbs ~80us barrier + delay)
        nc.gpsimd.collective_compute("AllReduce", OP.add, ins=[cc_dummy_in[:]],
                                     outs=[cc_dummy_out[:]], replica_groups=RG)

        cpool = _stk.enter_context(tc.tile_pool(name="consts", bufs=1))
        ones_t = cpool.tile([128, 1], BF16, tag="ones")
        nc.sync.dma_start(ones_t[:], ones128[:])
        ident_t = cpool.tile([128, 128], BF16, tag="ident")
        nc.sync.dma_start(ident_t[:], ident[:])
        ones_row = cpool.tile([1, 128], BF16, tag="onesrow")
        nc.sync.dma_start(ones_row[:], ones128[:].rearrange("p q -> q p"))
        ones_row_f = cpool.tile([1, 128], F32, tag="onesrowf")
        nc.vector.tensor_copy(ones_row_f[:], ones_row[:])
        sw_t = cpool.tile([1, 512], BF16, tag="sw")
        nc.sync.dma_start(sw_t[:], sw_in[:])
        convw_t = cpool.tile([128, 2 * KC], F32, tag="convw")
        nc.sync.dma_start(convw_t[:], convw[:])
        convb_t = cpool.tile([128, 2], F32, tag="convb")
        nc.sync.dma_start(convb_t[:], convb[:])
        xpw_t = cpool.tile([128, 2 * 96], BF16, tag="xpw")
        nc.sync.dma_start(xpw_t[:], xpw[:])
        dtw_t = cpool.tile([64, 256], BF16, tag="dtw")
        nc.sync.dma_start(dtw_t[:], dtw[:])
        dtb_t = cpool.tile([128, 2], F32, tag="dtb")
        nc.sync.dma_start(dtb_t[:], dtb[:])
        asc_t = cpool.tile([128, 2 * NST], F32, tag="asc")
        nc.sync.dma_start(asc_t[:], a_sc[:])
        dvec_t = cpool.tile([128, 2], F32, tag="dvec")
        nc.sync.dma_start(dvec_t[:], dvec[:])
        swfc_t = cpool.tile([1, HID], BF16, tag="swfc")
        nc.sync.dma_start(swfc_t[:], swfc[:])
        sbfc_t = cpool.tile([128, 32], F32, tag="sbfc")
        nc.sync.dma_start(sbfc_t[:], sbfc[:])
        pjb_t = cpool.tile([128, 8], F32, tag="pjb")
        nc.sync.dma_start(pjb_t[:], pjb[:])
        eps_t = cpool.tile([128, 1], F32, tag="eps")
        nc.vector.memset(eps_t[:], EPS)
        sbin_t = cpool.tile([128, 4], F32, tag="sbin")
        if ln1b_nonzero:
            nc.sync.dma_start(sbin_t[:], sb_in[:])
        xres_t = [cpool.tile([128, TOKC], F32, tag=f"xres{m}", name=f"xres_t{m}")
                  for m in range(8)]
        for m in range(8):
            nc.sync.dma_start(xres_t[m][:], xres[m * 128:(m + 1) * 128, :])

        # persistent mamba-phase activations (freed after A2A staging)
        _mstk = ExitStack()
        mpool = _mstk.enter_context(tc.tile_pool(name="mamba", bufs=1))
        _xzstk = ExitStack()
        xzpool = _xzstk.enter_context(tc.tile_pool(name="xz", bufs=1))
        xmp = [xzpool.tile([128, 2, 3 + L], BF16, tag=f"xmp{d}", name=f"xmp{d}")
               for d in range(2)]
        zt = [xzpool.tile([128, 2, L], BF16, tag=f"z{d}", name=f"zt{d}")
              for d in range(2)]
        zs = [mpool.tile([128, 2, L], BF16, tag=f"zs{d}", name=f"zs{d}")
              for d in range(2)]
        xs = [mpool.tile([128, 2, L], BF16, tag=f"xs{d}", name=f"xs{d}")
              for d in range(2)]
        dt_t = [mpool.tile([128, 2, L], F32, tag=f"dt{d}", name=f"dt_t{d}")
                for d in range(2)]
        dtx = [mpool.tile([128, 2, L], BF16, tag=f"dtx{d}", name=f"dtx{d}")
               for d in range(2)]
        y3 = [mpool.tile([128, 2, L], BF16, tag=f"y3{d}", name=f"y3_{d}")
              for d in range(2)]
        for d in range(2):
            nc.vector.memset(xmp[d][:, :, 0:3], 0.0)

        # ==== Phase 1-4: per-batch-half pipeline ====
        # stats(b) -> rstd(b) -> in_proj(b) -> conv(b) -> x_proj(b) -> AR(b) -> dt(b)
        with tc.tile_pool(name="ph1", bufs=1) as p1, \
             tc.tile_pool(name="ph1sq", bufs=3) as p1sq, \
             tc.tile_pool(name="ps_st", bufs=1, space="PSUM") as ps_st, \
             tc.tile_pool(name="ps_in", bufs=1, space="PSUM") as ps_in, \
             tc.tile_pool(name="ps_rb", bufs=1, space="PSUM") as ps_rb, \
             tc.tile_pool(name="conv", bufs=2) as cvp, \
             tc.tile_pool(name="xp", bufs=2) as xpp, \
             tc.tile_pool(name="ps_xp", bufs=1, space="PSUM") as ps_xp, \
             tc.tile_pool(name="dts", bufs=2) as dts:
            xt = [p1.tile([128, TOK], BF16, tag=f"xt{k}", name=f"xt{k}")
                  for k in range(8)]
            for b in range(2):
                for k in range(8):
                    nc.sync.dma_start(xt[k][:, b * L:(b + 1) * L],
                                      xT[k * 128:(k + 1) * 128, b * L:(b + 1) * L])
            win_t = p1.tile([128, 8 * 512], BF16, tag="win")
            nc.sync.dma_start(win_t[:], win[:])
            with tc.tile_pool(name="ps_wu", bufs=1, space="PSUM") as ps_wu:
                wu = ps_wu.tile([128, 128], F32, tag="wu")
                for w in range(40):
                    nc.tensor.matmul(wu[:], ident_t[:], ident_t[:],
                                     start=(w == 0), stop=(w == 39),
                                     skip_group_check=True)
            negs = [p1.tile([1, L], BF16, tag=f"negm{b}", name=f"negm{b}")
                    for b in range(2)]
            r_reps = [p1.tile([128, L], F32, tag="r_rep", name=f"r_rep{b}")
                      for b in range(2)]

            for b in range(2):
                # ---- LN1 stats for this half ----
                sum_sb = p1.tile([1, L], F32, tag="rows", bufs=3)
                sq_sb = p1.tile([1, L], F32, tag="rows", bufs=3)
                for ch in range(2):
                    sl = slice(b * L + ch * 512, b * L + (ch + 1) * 512)
                    dsl = slice(ch * 512, (ch + 1) * 512)
                    pss = ps_st.tile([1, 512], F32, tag="pstat", bufs=2)
                    for k in range(8):
                        nc.tensor.matmul(pss[:], ones_t[:], xt[k][:, sl],
                                         start=(k == 0), stop=(k == 7))
                    nc.vector.tensor_copy(sum_sb[:, dsl], pss[:])
                    psq = ps_st.tile([1, 512], F32, tag="pstat", bufs=2)
                    for k in range(8):
                        xq = p1sq.tile([128, 512], BF16, tag="xq")
                        nc.vector.tensor_tensor(xq[:], xt[k][:, sl],
                                                xt[k][:, sl], OP.mult)
                        nc.tensor.matmul(psq[:], ones_t[:], xq[:],
                                         start=(k == 0), stop=(k == 7))
                    nc.vector.tensor_copy(sq_sb[:, dsl], psq[:])
                m_neg = p1.tile([1, L], F32, tag="rows", bufs=3)
                nc.vector.tensor_scalar_mul(m_neg[:], sum_sb[:], -1.0 / E)
                nc.vector.tensor_copy(negs[b][:], m_neg[:])
                msq = p1.tile([1, L], F32, tag="rows", bufs=3)
                nc.vector.tensor_tensor(msq[:], m_neg[:], m_neg[:], OP.mult)
                var = p1.tile([1, L], F32, tag="rows", bufs=3)
                nc.vector.scalar_tensor_tensor(var[:], sq_sb[:], 1.0 / E,
                                               msq[:], OP.mult, OP.subtract)
                lnv = p1.tile([1, L], F32, tag="rows", bufs=3)
                nc.scalar.activation(lnv[:], var[:], AF.Ln, bias=eps_t[0:1, :])
                r_sb = p1.tile([1, L], F32, tag="rows", bufs=3)
                nc.scalar.activation(r_sb[:], lnv[:], AF.Exp, scale=-0.5)
                # broadcast rstd across partitions via a K=1 matmul
                for hh in range(2):
                    prb = ps_rb.tile([128, 512], F32, tag="prb")
                    nc.tensor.matmul(prb[:], ones_row_f[:],
                                     r_sb[:, hh * 512:(hh + 1) * 512],
                                     start=True, stop=True)
                    nc.vector.tensor_copy(r_reps[b][:, hh * 512:(hh + 1) * 512],
                                          prb[:])

                # ---- in_proj for this half ----
                for ch in range(2):
                    col = ch * 512
                    sl = slice(b * L + col, b * L + col + 512)
                    for mt in range(4):
                        ps = ps_in.tile([128, 512], F32, tag="ps", bufs=3)
                        for k in range(8):
                            nc.tensor.matmul(
                                ps[:],
                                win_t[:, k * 512 + mt * 128:k * 512 + (mt + 1) * 128],
                                xt[k][:, sl], start=(k == 0), stop=False)
                        nc.tensor.matmul(ps[:], sw_t[:, mt * 128:(mt + 1) * 128],
                                         negs[b][:, col:col + 512],
                                         start=False, stop=True)
                        if mt < 2:
                            dst = xmp[mt][:, b, 3 + col:3 + col + 512]
                        else:
                            dst = zt[mt - 2][:, b, col:col + 512]
                        if ln1b_nonzero:
                            tmp = p1sq.tile([128, 512], F32, tag="eptmp")
                            nc.vector.tensor_tensor(tmp[:], ps[:],
                                                    r_reps[b][:, col:col + 512],
                                                    OP.mult)
                            nc.scalar.activation(dst, tmp[:], AF.Identity,
                                                 bias=sbin_t[:, mt:mt + 1])
                        else:
                            nc.vector.tensor_tensor(dst, ps[:],
                                                    r_reps[b][:, col:col + 512],
                                                    OP.mult)

                # ---- conv + silu for this half ----
                for d in range(2):
                    acc0 = cvp.tile([128, L], BF16, tag="acc", bufs=3)
                    nc.vector.tensor_scalar_mul(acc0[:], xmp[d][:, b, 0:L],
                                                convw_t[:, d * KC:d * KC + 1])
                    for k in (1, 2, 3):
                        acc1 = cvp.tile([128, L], BF16, tag="acc", bufs=3)
                        nc.vector.scalar_tensor_tensor(
                            acc1[:], xmp[d][:, b, k:k + L],
                            convw_t[:, d * KC + k:d * KC + k + 1],
                            acc0[:], OP.mult, OP.add)
                        acc0 = acc1
                    nc.scalar.activation(xs[d][:, b, :], acc0[:], AF.Silu,
                                         bias=convb_t[:, d:d + 1])
                    nc.scalar.activation(zs[d][:, b, :], zt[d][:, b, :], AF.Silu)

                # ---- x_proj partial + AllReduce for this half ----
                xdblp = xpp.tile([96, L], F32, tag="xdblp")
                for ch in range(2):
                    col = ch * 512
                    psx = ps_xp.tile([96, 512], F32, tag="psx")
                    for k in range(2):
                        nc.tensor.matmul(psx[:], xpw_t[:, k * 96:(k + 1) * 96],
                                         xs[k][:, b, col:col + 512],
                                         start=(k == 0), stop=(k == 1))
                    nc.vector.tensor_copy(xdblp[:, col:col + 512], psx[:])
                nc.sync.dma_start(ar_ins[b][:], xdblp[:])
                nc.gpsimd.collective_compute("AllReduce", OP.add,
                                             ins=[ar_ins[b][:]],
                                             outs=[ar_outs[b][:]],
                                             replica_groups=RG)

            for b in range(2):
                dtr_b = dts.tile([64, L], BF16, tag="dtrb")
                nc.gpsimd.dma_start(dtr_b[:], ar_outs[b][0:64, :])
                nc.gpsimd.dma_start(bc_bfs[b][:], ar_outs[b][64:96, :])
                for mt in range(2):
                    dte_t = dts.tile([128, L], BF16, tag="dte")
                    for ch in range(2):
                        col = ch * 512
                        psd = ps_in.tile([128, 512], F32, tag="ps", bufs=3)
                        nc.tensor.matmul(psd[:],
                                         dtw_t[:, mt * 128:(mt + 1) * 128],
                                         dtr_b[:, col:col + 512],
                                         start=True, stop=True)
                        nc.scalar.activation(dte_t[:, col:col + 512],
                                             psd[:], AF.Exp,
                                             bias=dtb_t[:, mt:mt + 1])
                    nc.scalar.activation(dt_t[mt][:, b, :], dte_t[:],
                                         AF.Ln, bias=1.0)
                    dtxb = dts.tile([128, L], BF16, tag="dtxb")
                    nc.vector.tensor_copy(dtxb[:], dt_t[mt][:, b, :])
                    nc.vector.tensor_tensor(dtx[mt][:, b, :], dtxb[:],
                                            xs[mt][:, b, :], OP.mult)

        # ===== Phase 5: scan (d-outer) + per-d AllToAll + split-K out_proj ====
        _xzstk.close()
        opp = _stk.enter_context(tc.tile_pool(name="op", bufs=1, side="right"))
        r1 = [opp.tile([128, TOKC], F32, tag=f"r1_{m}", name=f"r1_{m}")
              for m in range(8)]
        r1b = [opp.tile([128, TOKC], BF16, tag=f"r1b{m}", name=f"r1b{m}")
               for m in range(8)]
        G = 4
        NG = NST // G
        with tc.tile_pool(name="ps_po", bufs=1, space="PSUM") as ps_po, \
             tc.tile_pool(name="wop", bufs=1) as wop:
            po = [ps_po.tile([128, 512], F32, tag=f"po{i}", name=f"po{i}")
                  for i in range(4)]
            yf = [[wop.tile([128, TOKC], BF16, tag=f"yf{d}_{i}",
                            name=f"yf{d}_{i}") for i in range(NC)]
                  for d in range(2)]
            _scanstk = ExitStack()
            pa = _scanstk.enter_context(tc.tile_pool(name="scan_a", bufs=2))
            pbh = _scanstk.enter_context(tc.tile_pool(name="scan_bh", bufs=3))
            pr = _scanstk.enter_context(tc.tile_pool(name="scan_r", bufs=3))
            py = _scanstk.enter_context(tc.tile_pool(name="scan_y", bufs=1))
            ps_y = _scanstk.enter_context(
                tc.tile_pool(name="ps_y", bufs=2, space="PSUM"))
            for d in range(2):
                for b in range(2):
                    psy = ps_y.tile([128, L], F32, tag="psy")
                    for g in range(NG):
                        a_t = pa.tile([128, G, L], BF16, tag="a")
                        for j in range(G):
                            n = g * G + j
                            nc.scalar.activation(
                                a_t[:, j, :], dt_t[d][:, b, :], AF.Exp,
                                scale=asc_t[:, d * NST + n:d * NST + n + 1])
                        nc.vector.memset(a_t[:, :, 0:1], 0.0)
                        brep = pr.tile([128, G, L], BF16, tag="bcr")
                        nc.sync.dma_start(brep[:],
                                          _rep0(bc_bfs[b][g * G:(g + 1) * G, :]))
                        bx = pbh.tile([128, G, L], BF16, tag="bxhc")
                        dslice = dtx[d][:, b, :]
                        dxb = bass.AP(dslice.tensor, dslice.offset,
                                      [list(dslice.ap[0]), [0, G], [1, L]])
                        nc.vector.tensor_tensor(bx[:], dxb, brep[:], OP.mult)
                        h_t = pbh.tile([128, G, L], BF16, tag="bxhc")
                        nc.vector.tensor_tensor_scan(
                            h_t[:].rearrange("p a b -> p (a b)"),
                            a_t[:].rearrange("p a b -> p (a b)"),
                            bx[:].rearrange("p a b -> p (a b)"),
                            0.0, OP.mult, OP.add)
                        crep = pr.tile([128, G, L], BF16, tag="bcr")
                        nc.sync.dma_start(crep[:],
                                          _rep0(bc_bfs[b][16 + g * G:16 + (g + 1) * G, :]))
                        hc = pbh.tile([128, G, L], BF16, tag="bxhc")
                        nc.vector.tensor_tensor(hc[:], h_t[:], crep[:], OP.mult)
                        for j in range(G):
                            for hh in range(2):
                                nc.tensor.matmul(
                                    psy[:, hh * 512:(hh + 1) * 512], ident_t[:],
                                    hc[:, j, hh * 512:(hh + 1) * 512],
                                    start=(g == 0 and j == 0),
                                    stop=(g == NG - 1 and j == G - 1))
                    y2 = py.tile([128, L], BF16, tag="y2")
                    nc.vector.scalar_tensor_tensor(y2[:], xs[d][:, b, :],
                                                   dvec_t[:, d:d + 1], psy[:],
                                                   OP.mult, OP.add)
                    nc.vector.tensor_tensor(y3[d][:, b, :], y2[:],
                                            zs[d][:, b, :], OP.mult)
                # ---- A2A for this d-half + out_proj half-K accumulation ----
                for c in range(NC):
                    nc.sync.dma_start(
                        a2a_ins[d][c].rearrange("(p q) -> p q", p=128),
                        y3[d][:, c // 4, (c % 4) * TOKC:(c % 4 + 1) * TOKC])
                nc.gpsimd.collective_compute("AllToAll", OP.bypass,
                                             ins=[a2a_ins[d][:]],
                                             outs=[a2a_outs[d][:]],
                                             replica_groups=RG)
                wo_h = wop.tile([128, 8 * 1024], BF16, tag="wo", name=f"wo_h{d}")
                nc.sync.dma_start(wo_h[:], wo[:, d * 8192:(d + 1) * 8192])
                for i in range(NC):
                    nc.sync.dma_start(
                        yf[d][i][:],
                        a2a_outs[d][i].rearrange("(p q) -> p q", p=128))
                for mt in range(8):
                    out_ap = po[mt // 2][:, (mt % 2) * 256:(mt % 2) * 256 + 256]
                    for i in range(NC):
                        nc.tensor.matmul(
                            out_ap,
                            wo_h[:, i * 1024 + mt * 128:i * 1024 + (mt + 1) * 128],
                            yf[d][i][:],
                            start=(d == 0 and i == 0 and mt % 2 == 0),
                            stop=(d == 1 and i == NC - 1),
                            skip_group_check=True)
            _scanstk.close()
            for mt in range(8):
                out_ap = po[mt // 2][:, (mt % 2) * 256:(mt % 2) * 256 + 256]
                nc.vector.tensor_tensor(r1[mt][:], out_ap, xres_t[mt][:], OP.add)
                nc.vector.tensor_copy(r1b[mt][:], r1[mt][:])
        _mstk.close()
        _mlppre = ExitStack()
        mlpp = _mlppre.enter_context(tc.tile_pool(name="mlp", bufs=1))
        _wfcstk = ExitStack()
        wfcp = _wfcstk.enter_context(tc.tile_pool(name="wfcp", bufs=1, side="right"))
        wfc_t = wfcp.tile([128, 8 * HID], BF16, tag="wfc")
        nc.sync.dma_start(wfc_t[:], wfc[:])

        # ================= Phase 6: LN2 + MLP (token-sharded) =================
        with ExitStack() as _s2:
            ps_s2 = _s2.enter_context(
                tc.tile_pool(name="ps_s2", bufs=1, space="PSUM"))
            sum2 = opp.tile([1, TOKC], F32, tag="sum2")
            sq2 = opp.tile([1, TOKC], F32, tag="sq2")
            ps2a = ps_s2.tile([1, TOKC], F32, tag="ps2a")
            for k in range(8):
                nc.tensor.matmul(ps2a[:], ones_t[:], r1b[k][:],
                                 start=(k == 0), stop=(k == 7))
            nc.vector.tensor_copy(sum2[:], ps2a[:])
            ps2b = ps_s2.tile([1, TOKC], F32, tag="ps2b")
            for k in range(8):
                q2 = opp.tile([128, TOKC], BF16, tag="q2")
                nc.vector.tensor_tensor(q2[:], r1b[k][:], r1b[k][:], OP.mult)
                nc.tensor.matmul(ps2b[:], ones_t[:], q2[:],
                                 start=(k == 0), stop=(k == 7))
            nc.vector.tensor_copy(sq2[:], ps2b[:])
            m2n = opp.tile([1, TOKC], F32, tag="m2n")
            nc.vector.tensor_scalar_mul(m2n[:], sum2[:], -1.0 / E)
            msq2 = opp.tile([1, TOKC], F32, tag="msq2")
            nc.vector.tensor_tensor(msq2[:], m2n[:], m2n[:], OP.mult)
            var2 = opp.tile([1, TOKC], F32, tag="var2")
            nc.vector.scalar_tensor_tensor(var2[:], sq2[:], 1.0 / E, msq2[:],
                                           OP.mult, OP.subtract)
            lnv2 = opp.tile([1, TOKC], F32, tag="lnv2")
            nc.scalar.activation(lnv2[:], var2[:], AF.Ln, bias=eps_t[0:1, :])
            r2_sb = opp.tile([1, TOKC], F32, tag="r2_sb")
            nc.scalar.activation(r2_sb[:], lnv2[:], AF.Exp, scale=-0.5)
            m2b = opp.tile([1, TOKC], BF16, tag="m2b")
            nc.vector.tensor_copy(m2b[:], m2n[:])
            pr2 = ps_s2.tile([128, TOKC], F32, tag="pr2")
            nc.tensor.matmul(pr2[:], ones_row_f[:],
                             r2_sb[:], start=True, stop=True)
            r2_rep = opp.tile([128, TOKC], F32, tag="r2rep")
            nc.vector.tensor_copy(r2_rep[:], pr2[:])

        with tc.tile_pool(name="fcep", bufs=3) as fcep:
            h1 = [mlpp.tile([128, TOKC], BF16, tag=f"h1_{m}", name=f"h1_{m}")
                  for m in range(32)]
            wpj_t = mlpp.tile([128, 32 * E], BF16, tag="wpj")
            nc.sync.dma_start(wpj_t[:], wpj[:])
            with tc.tile_pool(name="ps_f", bufs=6, space="PSUM") as ps_f:
                for mt in range(32):
                    psf = ps_f.tile([128, TOKC], F32, tag="psf")
                    for k in range(8):
                        nc.tensor.matmul(
                            psf[:],
                            wfc_t[:, k * HID + mt * 128:k * HID + (mt + 1) * 128],
                            r1b[k][:], start=(k == 0), stop=False)
                    nc.tensor.matmul(psf[:], swfc_t[:, mt * 128:(mt + 1) * 128],
                                     m2b[:], start=False, stop=True)
                    tmp = fcep.tile([128, TOKC], F32, tag="fctmp")
                    nc.vector.tensor_tensor(tmp[:], psf[:], r2_rep[:], OP.mult)
                    nc.scalar.activation(h1[mt][:], tmp[:], AF.Gelu,
                                         bias=sbfc_t[:, mt:mt + 1])
            _wfcstk.close()
            with tc.tile_pool(name="ps_p", bufs=1, space="PSUM") as ps_p, \
                 tc.tile_pool(name="pjep", bufs=2) as pjep:
                psps = [ps_p.tile([128, TOKC], F32, tag=f"psp{m}",
                                  name=f"psp{m}") for m in range(8)]
                for k in range(32):
                    for mt in range(8):
                        nc.tensor.matmul(psps[mt][:],
                                         wpj_t[:, k * E + mt * 128:k * E + (mt + 1) * 128],
                                         h1[k][:], start=(k == 0),
                                         stop=(k == 31))
                for mt in range(8):
                    ot = pjep.tile([128, TOKC], F32, tag="ot")
                    nc.vector.scalar_tensor_tensor(ot[:], psps[mt][:],
                                                   pjb_t[:, mt:mt + 1],
                                                   r1[mt][:], OP.add, OP.add)
                    nc.sync.dma_start(outT[mt * 128:(mt + 1) * 128, :], ot[:])
        _mlppre.close()

    nc.compile()
    _BUILD_CACHE[key] = nc
    return nc


def _prep_inputs(inputs):
    """Host-side sharding/packing. Returns list of per-core input dicts."""
    f32 = np.float32
    x = np.asarray(inputs["x"], f32)
    ln1_w = np.asarray(inputs["ln1_w"], f32)
    ln1_b = np.asarray(inputs["ln1_b"], f32)
    in_proj_w = np.asarray(inputs["in_proj_w"], f32)
    conv_w = np.asarray(inputs["conv_w"], f32)
    conv_b = np.asarray(inputs["conv_b"], f32)
    x_proj_w = np.asarray(inputs["x_proj_w"], f32)
    dt_proj_w = np.asarray(inputs["dt_proj_w"], f32)
    dt_proj_b = np.asarray(inputs["dt_proj_b"], f32)
    A_log = np.asarray(inputs["A_log"], f32)
    D = np.asarray(inputs["D"], f32)
    out_proj_w = np.asarray(inputs["out_proj_w"], f32)
    ln2_w = np.asarray(inputs["ln2_w"], f32)
    ln2_b = np.asarray(inputs["ln2_b"], f32)
    fc_w = np.asarray(inputs["fc_w"], f32)
    fc_b = np.asarray(inputs["fc_b"], f32)
    proj_w = np.asarray(inputs["proj_w"], f32)
    proj_b = np.asarray(inputs["proj_b"], f32)

    xT_f = np.ascontiguousarray(x.reshape(TOK, E).T)          # [E, TOK]
    xT_b = xT_f.astype(bf)

    def pack_lhsT(lhsT):
        K, M = lhsT.shape
        nk = K // 128
        return np.ascontiguousarray(
            lhsT.reshape(nk, 128, M).transpose(1, 0, 2).reshape(128, nk * M)
        ).astype(bf)

    Wp = in_proj_w * ln1_w[None, :]
    sb_full = in_proj_w @ ln1_b
    ln1b_nonzero = bool(np.any(sb_full != 0.0))

    Wfc = fc_w * ln2_w[None, :]
    swfc_full = Wfc.sum(1)
    sbfc_full = fc_w @ ln2_b + fc_b
    wfc_pack = pack_lhsT(np.ascontiguousarray(Wfc.T))
    wpj_pack = pack_lhsT(np.ascontiguousarray(proj_w.T))
    wo_pack0 = pack_lhsT(np.ascontiguousarray(out_proj_w.T))  # [128, 16*1024]
    blocks = [wo_pack0[:, k * 1024:(k + 1) * 1024] for k in range(16)]
    wo_pack = np.ascontiguousarray(
        np.concatenate([b for k, b in enumerate(blocks) if k % 2 == 0] +
                       [b for k, b in enumerate(blocks) if k % 2 == 1], axis=1))
    pjb_pack = np.ascontiguousarray(proj_b.reshape(8, 128).T).astype(f32)
    sbfc_pack = np.ascontiguousarray(sbfc_full.reshape(32, 128).T).astype(f32)
    swfc_row = swfc_full[None, :].astype(bf)

    A = -np.exp(A_log)

    per_core = []
    for c in range(NC):
        dsl = slice(c * DL, (c + 1) * DL)
        rows = np.concatenate([Wp[dsl], Wp[DIN + c * DL:DIN + (c + 1) * DL]])
        win_pack = pack_lhsT(np.ascontiguousarray(rows.T))
        sw_row = rows.sum(1)[None, :].astype(bf)
        sb_rows = np.concatenate([sb_full[dsl],
                                  sb_full[DIN + c * DL:DIN + (c + 1) * DL]])
        sb_pack = np.ascontiguousarray(sb_rows.reshape(4, 128).T).astype(f32)

        cw = conv_w[dsl, 0, :]
        convw_pack = np.ascontiguousarray(
            cw.reshape(2, 128, KC).transpose(1, 0, 2).reshape(128, 2 * KC)
        ).astype(f32)
        convb_pack = np.ascontiguousarray(
            conv_b[dsl].reshape(2, 128).T).astype(f32)

        xpw_pack = pack_lhsT(np.ascontiguousarray(x_proj_w[:, dsl].T))
        dtw_slice = np.ascontiguousarray(dt_proj_w[dsl].T).astype(bf)
        dtb_pack = np.ascontiguousarray(
            dt_proj_b[dsl].reshape(2, 128).T).astype(f32)
        asc_pack = np.ascontiguousarray(
            A[dsl].reshape(2, 128, NST).transpose(1, 0, 2).reshape(128, 2 * NST)
        ).astype(f32)
        dvec_pack = np.ascontiguousarray(D[dsl].reshape(2, 128).T).astype(f32)

        xres_slice = np.ascontiguousarray(xT_f[:, c * TOKC:(c + 1) * TOKC])

        per_core.append({
            "xT": xT_b, "win": win_pack, "sw_in": sw_row, "sb_in": sb_pack,
            "convw": convw_pack, "convb": convb_pack, "xpw": xpw_pack,
            "dtw": dtw_slice, "dtb": dtb_pack, "a_sc": asc_pack,
            "dvec": dvec_pack, "wo": wo_pack, "xres": xres_slice,
            "wfc": wfc_pack, "swfc": swfc_row, "sbfc": sbfc_pack,
            "wpj": wpj_pack, "pjb": pjb_pack,
            "ones128": np.ones((128, 1), bf),
            "ident": np.eye(128, dtype=bf),
        })
    return per_core, ln1b_nonzero


def kernel(**inputs):
    per_core, ln1b_nonzero = _prep_inputs(inputs)
    nc = _build(ln1b_nonzero)
    res = run_bass_kernel_spmd(
        nc, per_core, core_ids=list(range(NC)),
        trace=bool(int(os.environ.get("BASSK_TRACE", "0"))),
    )
    kernel.last_results = res
    out_T = np.concatenate([res.results[c]["outT"] for c in range(NC)], axis=1)
    return np.ascontiguousarray(out_T.T).reshape(B, L, E).astype(np.float32)


# revision 25
# speedup vs baseline: 1.0312x; 1.0312x over previous
"""Mamba block (dense_transformer nn_Block) on 8 Trainium2 NeuronCores.

Sharding: d_inner (2048 -> 256/core) for in_proj/conv/scan; per-batch-half
AllReduce for the small x_proj output; per-d-half AllToAll re-shards the scan
output to tokens (first half hidden under the second scan half); out_proj is a
split-K accumulation across the two A2A halves; MLP token-sharded. LayerNorms
fold into the following matmul (weights scaled host-side, mean correction via
a K=1 augmented matmul row, rstd applied in the PSUM epilogue).
"""
import os
import numpy as np
import ml_dtypes

import concourse.bass as bass
import concourse.bacc as bacc
import concourse.mybir as mybir
import concourse.tile as tile
from contextlib import ExitStack
from concourse.bass_utils import run_bass_kernel_spmd

BF16 = mybir.dt.bfloat16
F32 = mybir.dt.float32
AF = mybir.ActivationFunctionType
OP = mybir.AluOpType
bf = ml_dtypes.bfloat16

B, L, E = 2, 1024, 1024
DIN, NST, RDT, KC = 2 * E, 16, 64, 4
EPS = 1e-5
NC = 8
DL = DIN // NC          # 256 channels per core
TOK = B * L             # 2048
TOKC = TOK // NC        # 256 tokens per core post-A2A
HID = 4 * E             # 4096

_BUILD_CACHE = {}


def _rep0(src_ap, parts=128):
    """Partition-broadcast: prepend a [0, parts] dim to an AP's pattern."""
    return bass.AP(src_ap.tensor, src_ap.offset,
                   [[0, parts]] + [list(p) for p in src_ap.ap])


def _build(ln1b_nonzero):
    key = (ln1b_nonzero,)
    if key in _BUILD_CACHE:
        return _BUILD_CACHE[key]

    nc = bacc.Bacc("TRN2", target_bir_lowering=False, debug=False, num_devices=NC)

    def din(name, shape, dt=BF16):
        return nc.dram_tensor(name, shape, dt, kind="ExternalInput").ap()

    xT = din("xT", [E, TOK])
    win = din("win", [128, 8 * 512])
    sw_in = din("sw_in", [1, 512])
    sb_in = din("sb_in", [128, 4], F32)
    convw = din("convw", [128, 2 * KC], F32)
    convb = din("convb", [128, 2], F32)
    xpw = din("xpw", [128, 2 * 96])
    dtw = din("dtw", [64, 256])
    dtb = din("dtb", [128, 2], F32)
    a_sc = din("a_sc", [128, 2 * NST], F32)
    dvec = din("dvec", [128, 2], F32)
    wo = din("wo", [128, 16 * 1024])
    xres = din("xres", [E, TOKC], F32)
    wfc = din("wfc", [128, 8 * HID])
    swfc = din("swfc", [1, HID])
    sbfc = din("sbfc", [128, 32], F32)
    wpj = din("wpj", [128, 32 * E])
    pjb = din("pjb", [128, 8], F32)
    ones128 = din("ones128", [128, 1])
    ident = din("ident", [128, 128])

    outT = nc.dram_tensor("outT", [E, TOKC], F32, kind="ExternalOutput").ap()

    cc_dummy_in = nc.dram_tensor("cc_dummy_in", [1, 16], F32)
    cc_dummy_out = nc.dram_tensor("cc_dummy_out", [1, 16], F32, addr_space="Shared")
    ar_ins = [nc.dram_tensor(f"ar_in{b}", [96, L], F32) for b in range(2)]
    ar_outs = [nc.dram_tensor(f"ar_out{b}", [96, L], F32, addr_space="Shared")
               for b in range(2)]
    bc_bfs = [nc.dram_tensor(f"bc_bf{b}", [32, L], BF16) for b in range(2)]
    a2a_ins = [nc.dram_tensor(f"a2a_in{d}", [NC, 128 * TOKC], BF16)
               for d in range(2)]
    a2a_outs = [nc.dram_tensor(f"a2a_out{d}", [NC, 128 * TOKC], BF16)
                for d in range(2)]
    RG = [list(range(NC))]

    with tile.TileContext(nc) as tc, ExitStack() as _stk:
        # warm the collective stream early (absorbs ~80us barrier + delay)
        nc.gpsimd.collective_compute("AllReduce", OP.add, ins=[cc_dummy_in[:]],
                                     outs=[cc_dummy_out[:]], replica_groups=RG)

        cpool = _stk.enter_context(tc.tile_pool(name="consts", bufs=1))
        ones_t = cpool.tile([128, 1], BF16, tag="ones")
        nc.sync.dma_start(ones_t[:], ones128[:])
        ident_t = cpool.tile([128, 128], BF16, tag="ident")
        nc.sync.dma_start(ident_t[:], ident[:])
        ones_row = cpool.tile([1, 128], BF16, tag="onesrow")
        nc.sync.dma_start(ones_row[:], ones128[:].rearrange("p q -> q p"))
        ones_row_f = cpool.tile([1, 128], F32, tag="onesrowf")
        nc.vector.tensor_copy(ones_row_f[:], ones_row[:])
        sw_t = cpool.tile([1, 512], BF16, tag="sw")
        nc.sync.dma_start(sw_t[:], sw_in[:])
        convw_t = cpool.tile([128, 2 * KC], F32, tag="convw")
        nc.sync.dma_start(convw_t[:], convw[:])
        convb_t = cpool.tile([128, 2], F32, tag="convb")
        nc.sync.dma_start(convb_t[:], convb[:])
        xpw_t = cpool.tile([128, 2 * 96], BF16, tag="xpw")
        nc.sync.dma_start(xpw_t[:], xpw[:])
        dtw_t = cpool.tile([64, 256], BF16, tag="dtw")
        nc.sync.dma_start(dtw_t[:], dtw[:])
        dtb_t = cpool.tile([128, 2], F32, tag="dtb")
        nc.sync.dma_start(dtb_t[:], dtb[:])
        asc_t = cpool.tile([128, 2 * NST], F32, tag="asc")
        nc.sync.dma_start(asc_t[:], a_sc[:])
        dvec_t = cpool.tile([128, 2], F32, tag="dvec")
        nc.sync.dma_start(dvec_t[:], dvec[:])
        swfc_t = cpool.tile([1, HID], BF16, tag="swfc")
        nc.sync.dma_start(swfc_t[:], swfc[:])
        sbfc_t = cpool.tile([128, 32], F32, tag="sbfc")
        nc.sync.dma_start(sbfc_t[:], sbfc[:])
        pjb_t = cpool.tile([128, 8], F32, tag="pjb")
        nc.sync.dma_start(pjb_t[:], pjb[:])
        eps_t = cpool.tile([128, 1], F32, tag="eps")
        nc.vector.memset(eps_t[:], EPS)
        sbin_t = cpool.tile([128, 4], F32, tag="sbin")
        if ln1b_nonzero:
            nc.sync.dma_start(sbin_t[:], sb_in[:])
        xres_t = [cpool.tile([128, TOKC], F32, tag=f"xres{m}", name=f"xres_t{m}")
                  for m in range(8)]
        for m in range(8):
            nc.sync.dma_start(xres_t[m][:], xres[m * 128:(m + 1) * 128, :])

        # persistent mamba-phase activations (freed after A2A staging)
        _mstk = ExitStack()
        mpool = _mstk.enter_context(tc.tile_pool(name="mamba", bufs=1))
        _xzstk = ExitStack()
        xzpool = _xzstk.enter_context(tc.tile_pool(name="xz", bufs=1))
        xmp = [xzpool.tile([128, 2, 3 + L], BF16, tag=f"xmp{d}", name=f"xmp{d}")
               for d in range(2)]
        zt = [xzpool.tile([128, 2, L], BF16, tag=f"z{d}", name=f"zt{d}")
              for d in range(2)]
        zs = [mpool.tile([128, 2, L], BF16, tag=f"zs{d}", name=f"zs{d}")
              for d in range(2)]
        xs = [mpool.tile([128, 2, L], BF16, tag=f"xs{d}", name=f"xs{d}")
              for d in range(2)]
        dt_t = [mpool.tile([128, 2, L], F32, tag=f"dt{d}", name=f"dt_t{d}")
                for d in range(2)]
        dtx = [mpool.tile([128, 2, L], BF16, tag=f"dtx{d}", name=f"dtx{d}")
               for d in range(2)]
        y3 = [mpool.tile([128, 2, L], BF16, tag=f"y3{d}", name=f"y3_{d}")
              for d in range(2)]
        for d in range(2):
            nc.vector.memset(xmp[d][:, :, 0:3], 0.0)

        # ==== Phase 1-4: per-batch-half pipeline ====
        # stats(b) -> rstd(b) -> in_proj(b) -> conv(b) -> x_proj(b) -> AR(b) -> dt(b)
        with tc.tile_pool(name="ph1", bufs=1) as p1, \
             tc.tile_pool(name="ph1sq", bufs=3) as p1sq, \
             tc.tile_pool(name="ps_st", bufs=1, space="PSUM") as ps_st, \
             tc.tile_pool(name="ps_in", bufs=1, space="PSUM") as ps_in, \
             tc.tile_pool(name="ps_rb", bufs=1, space="PSUM") as ps_rb, \
             tc.tile_pool(name="conv", bufs=2) as cvp, \
             tc.tile_pool(name="xp", bufs=2) as xpp, \
             tc.tile_pool(name="ps_xp", bufs=1, space="PSUM") as ps_xp, \
             tc.tile_pool(name="dts", bufs=2) as dts:
            xt = [p1.tile([128, TOK], BF16, tag=f"xt{k}", name=f"xt{k}")
                  for k in range(8)]
            for b in range(2):
                for k in range(8):
                    nc.sync.dma_start(xt[k][:, b * L:(b + 1) * L],
                                      xT[k * 128:(k + 1) * 128, b * L:(b + 1) * L])
            win_t = p1.tile([128, 8 * 512], BF16, tag="win")
            nc.sync.dma_start(win_t[:], win[:])
            negs = [p1.tile([1, L], BF16, tag=f"negm{b}", name=f"negm{b}")
                    for b in range(2)]
            r_reps = [p1.tile([128, L], F32, tag="r_rep", name=f"r_rep{b}")
                      for b in range(2)]

            for b in range(2):
                # ---- LN1 stats for this half ----
                sum_sb = p1.tile([1, L], F32, tag="rows", bufs=3)
                sq_sb = p1.tile([1, L], F32, tag="rows", bufs=3)
                for ch in range(2):
                    sl = slice(b * L + ch * 512, b * L + (ch + 1) * 512)
                    dsl = slice(ch * 512, (ch + 1) * 512)
                    pss = ps_st.tile([1, 512], F32, tag="pstat", bufs=2)
                    for k in range(8):
                        nc.tensor.matmul(pss[:], ones_t[:], xt[k][:, sl],
                                         start=(k == 0), stop=(k == 7))
                    nc.vector.tensor_copy(sum_sb[:, dsl], pss[:])
                    psq = ps_st.tile([1, 512], F32, tag="pstat", bufs=2)
                    for k in range(8):
                        xq = p1sq.tile([128, 512], BF16, tag="xq")
                        nc.vector.tensor_tensor(xq[:], xt[k][:, sl],
                                                xt[k][:, sl], OP.mult)
                        nc.tensor.matmul(psq[:], ones_t[:], xq[:],
                                         start=(k == 0), stop=(k == 7))
                    nc.vector.tensor_copy(sq_sb[:, dsl], psq[:])
                m_neg = p1.tile([1, L], F32, tag="rows", bufs=3)
                nc.vector.tensor_scalar_mul(m_neg[:], sum_sb[:], -1.0 / E)
                nc.vector.tensor_copy(negs[b][:], m_neg[:])
                msq = p1.tile([1, L], F32, tag="rows", bufs=3)
                nc.vector.tensor_tensor(msq[:], m_neg[:], m_neg[:], OP.mult)
                var = p1.tile([1, L], F32, tag="rows", bufs=3)
                nc.vector.scalar_tensor_tensor(var[:], sq_sb[:], 1.0 / E,
                                               msq[:], OP.mult, OP.subtract)
                lnv = p1.tile([1, L], F32, tag="rows", bufs=3)
                nc.scalar.activation(lnv[:], var[:], AF.Ln, bias=eps_t[0:1, :])
                r_sb = p1.tile([1, L], F32, tag="rows", bufs=3)
                nc.scalar.activation(r_sb[:], lnv[:], AF.Exp, scale=-0.5)
                # broadcast rstd across partitions via a K=1 matmul
                for hh in range(2):
                    prb = ps_rb.tile([128, 512], F32, tag="prb")
                    nc.tensor.matmul(prb[:], ones_row_f[:],
                                     r_sb[:, hh * 512:(hh + 1) * 512],
                                     start=True, stop=True)
                    nc.vector.tensor_copy(r_reps[b][:, hh * 512:(hh + 1) * 512],
                                          prb[:])

                # ---- in_proj for this half ----
                for ch in range(2):
                    col = ch * 512
                    sl = slice(b * L + col, b * L + col + 512)
                    for mt in range(4):
                        ps = ps_in.tile([128, 512], F32, tag="ps", bufs=3)
                        for k in range(8):
                            nc.tensor.matmul(
                                ps[:],
                                win_t[:, k * 512 + mt * 128:k * 512 + (mt + 1) * 128],
                                xt[k][:, sl], start=(k == 0), stop=False)
                        nc.tensor.matmul(ps[:], sw_t[:, mt * 128:(mt + 1) * 128],
                                         negs[b][:, col:col + 512],
                                         start=False, stop=True)
                        if mt < 2:
                            dst = xmp[mt][:, b, 3 + col:3 + col + 512]
                        else:
                            dst = zt[mt - 2][:, b, col:col + 512]
                        if ln1b_nonzero:
                            tmp = p1sq.tile([128, 512], F32, tag="eptmp")
                            nc.vector.tensor_tensor(tmp[:], ps[:],
                                                    r_reps[b][:, col:col + 512],
                                                    OP.mult)
                            nc.scalar.activation(dst, tmp[:], AF.Identity,
                                                 bias=sbin_t[:, mt:mt + 1])
                        else:
                            nc.vector.tensor_tensor(dst, ps[:],
                                                    r_reps[b][:, col:col + 512],
                                                    OP.mult)

                # ---- conv + silu for this half ----
                for d in range(2):
                    acc0 = cvp.tile([128, L], BF16, tag="acc", bufs=3)
                    nc.vector.tensor_scalar_mul(acc0[:], xmp[d][:, b, 0:L],
                                                convw_t[:, d * KC:d * KC + 1])
                    for k in (1, 2, 3):
                        acc1 = cvp.tile([128, L], BF16, tag="acc", bufs=3)
                        nc.vector.scalar_tensor_tensor(
                            acc1[:], xmp[d][:, b, k:k + L],
                            convw_t[:, d * KC + k:d * KC + k + 1],
                            acc0[:], OP.mult, OP.add)
                        acc0 = acc1
                    nc.scalar.activation(xs[d][:, b, :], acc0[:], AF.Silu,
                                         bias=convb_t[:, d:d + 1])
                    nc.scalar.activation(zs[d][:, b, :], zt[d][:, b, :], AF.Silu)

                # ---- x_proj partial + AllReduce for this half ----
                xdblp = xpp.tile([96, L], F32, tag="xdblp")
                for ch in range(2):
                    col = ch * 512
                    psx = ps_xp.tile([96, 512], F32, tag="psx")
                    for k in range(2):
                        nc.tensor.matmul(psx[:], xpw_t[:, k * 96:(k + 1) * 96],
                                         xs[k][:, b, col:col + 512],
                                         start=(k == 0), stop=(k == 1))
                    nc.vector.tensor_copy(xdblp[:, col:col + 512], psx[:])
                nc.sync.dma_start(ar_ins[b][:], xdblp[:])
                nc.gpsimd.collective_compute("AllReduce", OP.add,
                                             ins=[ar_ins[b][:]],
                                             outs=[ar_outs[b][:]],
                                             replica_groups=RG)

            for b in range(2):
                dtr_b = dts.tile([64, L], BF16, tag="dtrb")
                nc.gpsimd.dma_start(dtr_b[:], ar_outs[b][0:64, :])
                nc.gpsimd.dma_start(bc_bfs[b][:], ar_outs[b][64:96, :])
                for mt in range(2):
                    dte_t = dts.tile([128, L], BF16, tag="dte")
                    for ch in range(2):
                        col = ch * 512
                        psd = ps_in.tile([128, 512], F32, tag="ps", bufs=3)
                        nc.tensor.matmul(psd[:],
                                         dtw_t[:, mt * 128:(mt + 1) * 128],
                                         dtr_b[:, col:col + 512],
                                         start=True, stop=True)
                        nc.scalar.activation(dte_t[:, col:col + 512],
                                             psd[:], AF.Exp,
                                             bias=dtb_t[:, mt:mt + 1])
                    nc.scalar.activation(dt_t[mt][:, b, :], dte_t[:],
                                         AF.Ln, bias=1.0)
                    dtxb = dts.tile([128, L], BF16, tag="dtxb")
                    nc.vector.tensor_copy(dtxb[:], dt_t[mt][:, b, :])
                    nc.vector.tensor_tensor(dtx[mt][:, b, :], dtxb[:],
                                            xs[mt][:, b, :], OP.mult)

        # ===== Phase 5: scan (d-outer) + per-d AllToAll + split-K out_proj ====
        _xzstk.close()
        opp = _stk.enter_context(tc.tile_pool(name="op", bufs=1, side="right"))
        r1 = [opp.tile([128, TOKC], F32, tag=f"r1_{m}", name=f"r1_{m}")
              for m in range(8)]
        r1b = [opp.tile([128, TOKC], BF16, tag=f"r1b{m}", name=f"r1b{m}")
               for m in range(8)]
        G = 4
        NG = NST // G
        with tc.tile_pool(name="ps_po", bufs=1, space="PSUM") as ps_po, \
             tc.tile_pool(name="wop", bufs=1) as wop:
            po = [ps_po.tile([128, 512], F32, tag=f"po{i}", name=f"po{i}")
                  for i in range(4)]
            yf = [[wop.tile([128, TOKC], BF16, tag=f"yf{d}_{i}",
                            name=f"yf{d}_{i}") for i in range(NC)]
                  for d in range(2)]
            _scanstk = ExitStack()
            pa = _scanstk.enter_context(tc.tile_pool(name="scan_a", bufs=2))
            pbh = _scanstk.enter_context(tc.tile_pool(name="scan_bh", bufs=3))
            pr = _scanstk.enter_context(tc.tile_pool(name="scan_r", bufs=3))
            py = _scanstk.enter_context(tc.tile_pool(name="scan_y", bufs=1))
            ps_y = _scanstk.enter_context(
                tc.tile_pool(name="ps_y", bufs=2, space="PSUM"))
            for d in range(2):
                for b in range(2):
                    psy = ps_y.tile([128, L], F32, tag="psy")
                    for g in range(NG):
                        a_t = pa.tile([128, G, L], BF16, tag="a")
                        for j in range(G):
                            n = g * G + j
                            nc.scalar.activation(
                                a_t[:, j, :], dt_t[d][:, b, :], AF.Exp,
                                scale=asc_t[:, d * NST + n:d * NST + n + 1])
                        nc.vector.memset(a_t[:, :, 0:1], 0.0)
                        brep = pr.tile([128, G, L], BF16, tag="bcr")
                        nc.sync.dma_start(brep[:],
                                          _rep0(bc_bfs[b][g * G:(g + 1) * G, :]))
                        bx = pbh.tile([128, G, L], BF16, tag="bxhc")
                        dslice = dtx[d][:, b, :]
                        dxb = bass.AP(dslice.tensor, dslice.offset,
                                      [list(dslice.ap[0]), [0, G], [1, L]])
                        nc.vector.tensor_tensor(bx[:], dxb, brep[:], OP.mult)
                        h_t = pbh.tile([128, G, L], BF16, tag="bxhc")
                        nc.vector.tensor_tensor_scan(
                            h_t[:].rearrange("p a b -> p (a b)"),
                            a_t[:].rearrange("p a b -> p (a b)"),
                            bx[:].rearrange("p a b -> p (a b)"),
                            0.0, OP.mult, OP.add)
                        crep = pr.tile([128, G, L], BF16, tag="bcr")
                        nc.sync.dma_start(crep[:],
                                          _rep0(bc_bfs[b][16 + g * G:16 + (g + 1) * G, :]))
                        hc = pbh.tile([128, G, L], BF16, tag="bxhc")
                        nc.vector.tensor_tensor(hc[:], h_t[:], crep[:], OP.mult)
                        for j in range(G):
                            for hh in range(2):
                                nc.tensor.matmul(
                                    psy[:, hh * 512:(hh + 1) * 512], ident_t[:],
                                    hc[:, j, hh * 512:(hh + 1) * 512],
                                    start=(g == 0 and j == 0),
                                    stop=(g == NG - 1 and j == G - 1))
                    y2 = py.tile([128, L], BF16, tag="y2")
                    nc.vector.scalar_tensor_tensor(y2[:], xs[d][:, b, :],
                                                   dvec_t[:, d:d + 1], psy[:],
                                                   OP.mult, OP.add)
                    nc.vector.tensor_tensor(y3[d][:, b, :], y2[:],
                                            zs[d][:, b, :], OP.mult)
                # ---- A2A for this d-half + out_proj half-K accumulation ----
                for c in range(NC):
                    nc.sync.dma_start(
                        a2a_ins[d][c].rearrange("(p q) -> p q", p=128),
                        y3[d][:, c // 4, (c % 4) * TOKC:(c % 4 + 1) * TOKC])
                nc.gpsimd.collective_compute("AllToAll", OP.bypass,
                                             ins=[a2a_ins[d][:]],
                                             outs=[a2a_outs[d][:]],
                                             replica_groups=RG)
                wo_h = wop.tile([128, 8 * 1024], BF16, tag="wo", name=f"wo_h{d}")
                nc.sync.dma_start(wo_h[:], wo[:, d * 8192:(d + 1) * 8192])
                for i in range(NC):
                    nc.sync.dma_start(
                        yf[d][i][:],
                        a2a_outs[d][i].rearrange("(p q) -> p q", p=128))
                for mt in range(8):
                    out_ap = po[mt // 2][:, (mt % 2) * 256:(mt % 2) * 256 + 256]
                    for i in range(NC):
                        nc.tensor.matmul(
                            out_ap,
                            wo_h[:, i * 1024 + mt * 128:i * 1024 + (mt + 1) * 128],
                            yf[d][i][:],
                            start=(d == 0 and i == 0 and mt % 2 == 0),
                            stop=(d == 1 and i == NC - 1),
                            skip_group_check=True)
            _scanstk.close()
            for mt in range(8):
                out_ap = po[mt // 2][:, (mt % 2) * 256:(mt % 2) * 256 + 256]
                nc.vector.tensor_tensor(r1[mt][:], out_ap, xres_t[mt][:], OP.add)
                nc.vector.tensor_copy(r1b[mt][:], r1[mt][:])
        _mstk.close()
        _mlppre = ExitStack()
        mlpp = _mlppre.enter_context(tc.tile_pool(name="mlp", bufs=1))
        _wfcstk = ExitStack()
        wfcp = _wfcstk.enter_context(tc.tile_pool(name="wfcp", bufs=1, side="right"))
        wfc_t = wfcp.tile([128, 8 * HID], BF16, tag="wfc")
        nc.sync.dma_start(wfc_t[:], wfc[:])

        # ================= Phase 6: LN2 + MLP (token-sharded) =================
        with ExitStack() as _s2:
            ps_s2 = _s2.enter_context(
                tc.tile_pool(name="ps_s2", bufs=1, space="PSUM"))
            sum2 = opp.tile([1, TOKC], F32, tag="sum2")
            sq2 = opp.tile([1, TOKC], F32, tag="sq2")
            ps2a = ps_s2.tile([1, TOKC], F32, tag="ps2a")
            for k in range(8):
                nc.tensor.matmul(ps2a[:], ones_t[:], r1b[k][:],
                                 start=(k == 0), stop=(k == 7))
            nc.vector.tensor_copy(sum2[:], ps2a[:])
            ps2b = ps_s2.tile([1, TOKC], F32, tag="ps2b")
            for k in range(8):
                q2 = opp.tile([128, TOKC], BF16, tag="q2")
                nc.vector.tensor_tensor(q2[:], r1b[k][:], r1b[k][:], OP.mult)
                nc.tensor.matmul(ps2b[:], ones_t[:], q2[:],
                                 start=(k == 0), stop=(k == 7))
            nc.vector.tensor_copy(sq2[:], ps2b[:])
            m2n = opp.tile([1, TOKC], F32, tag="m2n")
            nc.vector.tensor_scalar_mul(m2n[:], sum2[:], -1.0 / E)
            msq2 = opp.tile([1, TOKC], F32, tag="msq2")
            nc.vector.tensor_tensor(msq2[:], m2n[:], m2n[:], OP.mult)
            var2 = opp.tile([1, TOKC], F32, tag="var2")
            nc.vector.scalar_tensor_tensor(var2[:], sq2[:], 1.0 / E, msq2[:],
                                           OP.mult, OP.subtract)
            lnv2 = opp.tile([1, TOKC], F32, tag="lnv2")
            nc.scalar.activation(lnv2[:], var2[:], AF.Ln, bias=eps_t[0:1, :])
            r2_sb = opp.tile([1, TOKC], F32, tag="r2_sb")
            nc.scalar.activation(r2_sb[:], lnv2[:], AF.Exp, scale=-0.5)
            m2b = opp.tile([1, TOKC], BF16, tag="m2b")
            nc.vector.tensor_copy(m2b[:], m2n[:])
            pr2 = ps_s2.tile([128, TOKC], F32, tag="pr2")
            nc.tensor.matmul(pr2[:], ones_row_f[:],
                             r2_sb[:], start=True, stop=True)
            r2_rep = opp.tile([128, TOKC], F32, tag="r2rep")
            nc.vector.tensor_copy(r2_rep[:], pr2[:])

        with tc.tile_pool(name="fcep", bufs=3) as fcep:
            h1 = [mlpp.tile([128, TOKC], BF16, tag=f"h1_{m}", name=f"h1_{m}")
                  for m in range(32)]
            wpj_t = mlpp.tile([128, 32 * E], BF16, tag="wpj")
            nc.sync.dma_start(wpj_t[:], wpj[:])
            with tc.tile_pool(name="ps_f", bufs=6, space="PSUM") as ps_f:
                for mt in range(32):
                    psf = ps_f.tile([128, TOKC], F32, tag="psf")
                    for k in range(8):
                        nc.tensor.matmul(
                            psf[:],
                            wfc_t[:, k * HID + mt * 128:k * HID + (mt + 1) * 128],
                            r1b[k][:], start=(k == 0), stop=False)
                    nc.tensor.matmul(psf[:], swfc_t[:, mt * 128:(mt + 1) * 128],
                                     m2b[:], start=False, stop=True)
                    tmp = fcep.tile([128, TOKC], F32, tag="fctmp")
                    nc.vector.tensor_tensor(tmp[:], psf[:], r2_rep[:], OP.mult)
                    nc.scalar.activation(h1[mt][:], tmp[:], AF.Gelu,
                                         bias=sbfc_t[:, mt:mt + 1])
            _wfcstk.close()
            with tc.tile_pool(name="ps_p", bufs=1, space="PSUM") as ps_p, \
                 tc.tile_pool(name="pjep", bufs=2) as pjep:
                psps = [ps_p.tile([128, TOKC], F32, tag=f"psp{m}",
                                  name=f"psp{m}") for m in range(8)]
                for k in range(32):
                    for mt in range(8):
                        nc.tensor.matmul(psps[mt][:],
                                         wpj_t[:, k * E + mt * 128:k * E + (mt + 1) * 128],
                                         h1[k][:], start=(k == 0),
                                         stop=(k == 31))
                for mt in range(8):
                    ot = pjep.tile([128, TOKC], F32, tag="ot")
                    nc.vector.scalar_tensor_tensor(ot[:], psps[mt][:],
                                                   pjb_t[:, mt:mt + 1],
                                                   r1[mt][:], OP.add, OP.add)
                    nc.sync.dma_start(outT[mt * 128:(mt + 1) * 128, :], ot[:])
        _mlppre.close()

    nc.compile()
    _BUILD_CACHE[key] = nc
    return nc


def _prep_inputs(inputs):
    """Host-side sharding/packing. Returns list of per-core input dicts."""
    f32 = np.float32
    x = np.asarray(inputs["x"], f32)
    ln1_w = np.asarray(inputs["ln1_w"], f32)
    ln1_b = np.asarray(inputs["ln1_b"], f32)
    in_proj_w = np.asarray(inputs["in_proj_w"], f32)
    conv_w = np.asarray(inputs["conv_w"], f32)
    conv_b = np.asarray(inputs["conv_b"], f32)
    x_proj_w = np.asarray(inputs["x_proj_w"], f32)
    dt_proj_w = np.asarray(inputs["dt_proj_w"], f32)
    dt_proj_b = np.asarray(inputs["dt_proj_b"], f32)
    A_log = np.asarray(inputs["A_log"], f32)
    D = np.asarray(inputs["D"], f32)
    out_proj_w = np.asarray(inputs["out_proj_w"], f32)
    ln2_w = np.asarray(inputs["ln2_w"], f32)
    ln2_b = np.asarray(inputs["ln2_b"], f32)
    fc_w = np.asarray(inputs["fc_w"], f32)
    fc_b = np.asarray(inputs["fc_b"], f32)
    proj_w = np.asarray(inputs["proj_w"], f32)
    proj_b = np.asarray(inputs["proj_b"], f32)

    xT_f = np.ascontiguousarray(x.reshape(TOK, E).T)          # [E, TOK]
    xT_b = xT_f.astype(bf)

    def pack_lhsT(lhsT):
        K, M = lhsT.shape
        nk = K // 128
        return np.ascontiguousarray(
            lhsT.reshape(nk, 128, M).transpose(1, 0, 2).reshape(128, nk * M)
        ).astype(bf)

    Wp = in_proj_w * ln1_w[None, :]
    sb_full = in_proj_w @ ln1_b
    ln1b_nonzero = bool(np.any(sb_full != 0.0))

    Wfc = fc_w * ln2_w[None, :]
    swfc_full = Wfc.sum(1)
    sbfc_full = fc_w @ ln2_b + fc_b
    wfc_pack = pack_lhsT(np.ascontiguousarray(Wfc.T))
    wpj_pack = pack_lhsT(np.ascontiguousarray(proj_w.T))
    wo_pack0 = pack_lhsT(np.ascontiguousarray(out_proj_w.T))  # [128, 16*1024]
    blocks = [wo_pack0[:, k * 1024:(k + 1) * 1024] for k in range(16)]
    wo_pack = np.ascontiguousarray(
        np.concatenate([b for k, b in enumerate(blocks) if k % 2 == 0] +
                       [b for k, b in enumerate(blocks) if k % 2 == 1], axis=1))
    pjb_pack = np.ascontiguousarray(proj_b.reshape(8, 128).T).astype(f32)
    sbfc_pack = np.ascontiguousarray(sbfc_full.reshape(32, 128).T).astype(f32)
    swfc_row = swfc_full[None, :].astype(bf)

    A = -np.exp(A_log)

    per_core = []
    for c in range(NC):
        dsl = slice(c * DL, (c + 1) * DL)
        rows = np.concatenate([Wp[dsl], Wp[DIN + c * DL:DIN + (c + 1) * DL]])
        win_pack = pack_lhsT(np.ascontiguousarray(rows.T))
        sw_row = rows.sum(1)[None, :].astype(bf)
        sb_rows = np.concatenate([sb_full[dsl],
                                  sb_full[DIN + c * DL:DIN + (c + 1) * DL]])
        sb_pack = np.ascontiguousarray(sb_rows.reshape(4, 128).T).astype(f32)

        cw = conv_w[dsl, 0, :]
        convw_pack = np.ascontiguousarray(
            cw.reshape(2, 128, KC).transpose(1, 0, 2).reshape(128, 2 * KC)
        ).astype(f32)
        convb_pack = np.ascontiguousarray(
            conv_b[dsl].reshape(2, 128).T).astype(f32)

        xpw_pack = pack_lhsT(np.ascontiguousarray(x_proj_w[:, dsl].T))
        dtw_slice = np.ascontiguousarray(dt_proj_w[dsl].T).astype(bf)
        dtb_pack = np.ascontiguousarray(
            dt_proj_b[dsl].reshape(2, 128).T).astype(f32)
        asc_pack = np.ascontiguousarray(
            A[dsl].reshape(2, 128, NST).transpose(1, 0, 2).reshape(128, 2 * NST)
        ).astype(f32)
        dvec_pack = np.ascontiguousarray(D[dsl].reshape(2, 128).T).astype(f32)

        xres_slice = np.ascontiguousarray(xT_f[:, c * TOKC:(c + 1) * TOKC])

        per_core.append({
            "xT": xT_b, "win": win_pack, "sw_in": sw_row, "sb_in": sb_pack,
            "convw": convw_pack, "convb": convb_pack, "xpw": xpw_pack,
            "dtw": dtw_slice, "dtb": dtb_pack, "a_sc": asc_pack,
            "dvec": dvec_pack, "wo": wo_pack, "xres": xres_slice,
            "wfc": wfc_pack, "swfc": swfc_row, "sbfc": sbfc_pack,
            "wpj": wpj_pack, "pjb": pjb_pack,
            "ones128": np.ones((128, 1), bf),
            "ident": np.eye(128, dtype=bf),
        })
    return per_core, ln1b_nonzero


def kernel(**inputs):
    per_core, ln1b_nonzero = _prep_inputs(inputs)
    nc = _build(ln1b_nonzero)
    res = run_bass_kernel_spmd(
        nc, per_core, core_ids=list(range(NC)),
        trace=bool(int(os.environ.get("BASSK_TRACE", "0"))),
    )
    kernel.last_results = res
    out_T = np.concatenate([res.results[c]["outT"] for c in range(NC)], axis=1)
    return np.ascontiguousarray(out_T.T).reshape(B, L, E).astype(np.float32)


# revision 26
# speedup vs baseline: 1.0429x; 1.0114x over previous
"""Mamba block (dense_transformer nn_Block) on 8 Trainium2 NeuronCores.

Sharding: d_inner (2048 -> 256/core) for in_proj/conv/scan; per-batch-half
AllReduce for the small x_proj output; per-d-half AllToAll re-shards the scan
output to tokens (first half hidden under the second scan half); out_proj is a
split-K accumulation across the two A2A halves; MLP token-sharded. LayerNorms
fold into the following matmul (weights scaled host-side, mean correction via
a K=1 augmented matmul row, rstd applied in the PSUM epilogue).
"""
import os
import numpy as np
import ml_dtypes

import concourse.bass as bass
import concourse.bacc as bacc
import concourse.mybir as mybir
import concourse.tile as tile
from contextlib import ExitStack
from concourse.bass_utils import run_bass_kernel_spmd

BF16 = mybir.dt.bfloat16
F32 = mybir.dt.float32
AF = mybir.ActivationFunctionType
OP = mybir.AluOpType
bf = ml_dtypes.bfloat16

B, L, E = 2, 1024, 1024
DIN, NST, RDT, KC = 2 * E, 16, 64, 4
EPS = 1e-5
NC = 8
DL = DIN // NC          # 256 channels per core
TOK = B * L             # 2048
TOKC = TOK // NC        # 256 tokens per core post-A2A
HID = 4 * E             # 4096

_BUILD_CACHE = {}


def _rep0(src_ap, parts=128):
    """Partition-broadcast: prepend a [0, parts] dim to an AP's pattern."""
    return bass.AP(src_ap.tensor, src_ap.offset,
                   [[0, parts]] + [list(p) for p in src_ap.ap])


def _build(ln1b_nonzero):
    key = (ln1b_nonzero,)
    if key in _BUILD_CACHE:
        return _BUILD_CACHE[key]

    nc = bacc.Bacc("TRN2", target_bir_lowering=False, debug=False, num_devices=NC)

    def din(name, shape, dt=BF16):
        return nc.dram_tensor(name, shape, dt, kind="ExternalInput").ap()

    xT = din("xT", [E, TOK])
    win = din("win", [128, 8 * 512])
    sw_in = din("sw_in", [1, 512])
    sb_in = din("sb_in", [128, 4], F32)
    convw = din("convw", [128, 2 * KC], F32)
    convb = din("convb", [128, 2], F32)
    xpw = din("xpw", [128, 2 * 96])
    dtw = din("dtw", [64, 256])
    dtb = din("dtb", [128, 2], F32)
    a_sc = din("a_sc", [128, 2 * NST], F32)
    dvec = din("dvec", [128, 2], F32)
    wo = din("wo", [128, 16 * 1024])
    xres = din("xres", [E, TOKC], F32)
    wfc = din("wfc", [128, 8 * HID])
    swfc = din("swfc", [1, HID])
    sbfc = din("sbfc", [128, 32], F32)
    wpj = din("wpj", [128, 32 * E])
    pjb = din("pjb", [128, 8], F32)
    ones128 = din("ones128", [128, 1])
    ident = din("ident", [128, 128])

    outT = nc.dram_tensor("outT", [E, TOKC], F32, kind="ExternalOutput").ap()

    cc_dummy_in = nc.dram_tensor("cc_dummy_in", [1, 16], F32)
    cc_dummy_out = nc.dram_tensor("cc_dummy_out", [1, 16], F32, addr_space="Shared")
    ar_ins = [nc.dram_tensor(f"ar_in{b}", [96, L], F32) for b in range(2)]
    ar_outs = [nc.dram_tensor(f"ar_out{b}", [96, L], F32, addr_space="Shared")
               for b in range(2)]
    bc_bfs = [nc.dram_tensor(f"bc_bf{b}", [32, L], BF16) for b in range(2)]
    a2a_ins = [nc.dram_tensor(f"a2a_in{d}", [NC, 128 * TOKC], BF16)
               for d in range(2)]
    a2a_outs = [nc.dram_tensor(f"a2a_out{d}", [NC, 128 * TOKC], BF16)
                for d in range(2)]
    RG = [list(range(NC))]

    with tile.TileContext(nc) as tc, ExitStack() as _stk:
        # warm the collective stream early (absorbs ~80us barrier + delay)
        nc.gpsimd.collective_compute("AllReduce", OP.add, ins=[cc_dummy_in[:]],
                                     outs=[cc_dummy_out[:]], replica_groups=RG)

        cpool = _stk.enter_context(tc.tile_pool(name="consts", bufs=1))
        ones_t = cpool.tile([128, 1], BF16, tag="ones")
        nc.sync.dma_start(ones_t[:], ones128[:])
        ident_t = cpool.tile([128, 128], BF16, tag="ident")
        nc.sync.dma_start(ident_t[:], ident[:])
        ones_row = cpool.tile([1, 128], BF16, tag="onesrow")
        nc.sync.dma_start(ones_row[:], ones128[:].rearrange("p q -> q p"))
        ones_row_f = cpool.tile([1, 128], F32, tag="onesrowf")
        nc.vector.tensor_copy(ones_row_f[:], ones_row[:])
        sw_t = cpool.tile([1, 512], BF16, tag="sw")
        nc.sync.dma_start(sw_t[:], sw_in[:])
        convw_t = cpool.tile([128, 2 * KC], F32, tag="convw")
        nc.sync.dma_start(convw_t[:], convw[:])
        convb_t = cpool.tile([128, 2], F32, tag="convb")
        nc.sync.dma_start(convb_t[:], convb[:])
        xpw_t = cpool.tile([128, 2 * 96], BF16, tag="xpw")
        nc.sync.dma_start(xpw_t[:], xpw[:])
        dtw_t = cpool.tile([64, 256], BF16, tag="dtw")
        nc.sync.dma_start(dtw_t[:], dtw[:])
        dtb_t = cpool.tile([128, 2], F32, tag="dtb")
        nc.sync.dma_start(dtb_t[:], dtb[:])
        asc_t = cpool.tile([128, 2 * NST], F32, tag="asc")
        nc.sync.dma_start(asc_t[:], a_sc[:])
        dvec_t = cpool.tile([128, 2], F32, tag="dvec")
        nc.sync.dma_start(dvec_t[:], dvec[:])
        swfc_t = cpool.tile([1, HID], BF16, tag="swfc")
        nc.sync.dma_start(swfc_t[:], swfc[:])
        sbfc_t = cpool.tile([128, 32], F32, tag="sbfc")
        nc.sync.dma_start(sbfc_t[:], sbfc[:])
        pjb_t = cpool.tile([128, 8], F32, tag="pjb")
        nc.sync.dma_start(pjb_t[:], pjb[:])
        eps_t = cpool.tile([128, 1], F32, tag="eps")
        nc.vector.memset(eps_t[:], EPS)
        sbin_t = cpool.tile([128, 4], F32, tag="sbin")
        if ln1b_nonzero:
            nc.sync.dma_start(sbin_t[:], sb_in[:])
        xres_t = [cpool.tile([128, TOKC], F32, tag=f"xres{m}", name=f"xres_t{m}")
                  for m in range(8)]
        for m in range(8):
            nc.sync.dma_start(xres_t[m][:], xres[m * 128:(m + 1) * 128, :])

        # persistent mamba-phase activations (freed after A2A staging)
        _mstk = ExitStack()
        mpool = _mstk.enter_context(tc.tile_pool(name="mamba", bufs=1))
        _xzstk = ExitStack()
        xzpool = _xzstk.enter_context(tc.tile_pool(name="xz", bufs=1))
        xmp = [xzpool.tile([128, 2, 3 + L], BF16, tag=f"xmp{d}", name=f"xmp{d}")
               for d in range(2)]
        zt = [xzpool.tile([128, 2, L], BF16, tag=f"z{d}", name=f"zt{d}")
              for d in range(2)]
        zs = [mpool.tile([128, 2, L], BF16, tag=f"zs{d}", name=f"zs{d}")
              for d in range(2)]
        xs = [mpool.tile([128, 2, L], BF16, tag=f"xs{d}", name=f"xs{d}")
              for d in range(2)]
        dt_t = [mpool.tile([128, 2, L], F32, tag=f"dt{d}", name=f"dt_t{d}")
                for d in range(2)]
        dtx = [mpool.tile([128, 2, L], BF16, tag=f"dtx{d}", name=f"dtx{d}")
               for d in range(2)]
        y3 = [mpool.tile([128, 2, L], BF16, tag=f"y3{d}", name=f"y3_{d}")
              for d in range(2)]
        for d in range(2):
            nc.vector.memset(xmp[d][:, :, 0:3], 0.0)

        # ==== Phase 1-4: per-batch-half pipeline ====
        # stats(b) -> rstd(b) -> in_proj(b) -> conv(b) -> x_proj(b) -> AR(b) -> dt(b)
        with tc.tile_pool(name="ph1", bufs=1) as p1, \
             tc.tile_pool(name="ph1sq", bufs=3) as p1sq, \
             tc.tile_pool(name="ps_st", bufs=1, space="PSUM") as ps_st, \
             tc.tile_pool(name="ps_in", bufs=1, space="PSUM") as ps_in, \
             tc.tile_pool(name="ps_rb", bufs=1, space="PSUM") as ps_rb, \
             tc.tile_pool(name="conv", bufs=2) as cvp, \
             tc.tile_pool(name="xp", bufs=2) as xpp, \
             tc.tile_pool(name="ps_xp", bufs=1, space="PSUM") as ps_xp, \
             tc.tile_pool(name="dts", bufs=2) as dts:
            xt = [p1.tile([128, TOK], BF16, tag=f"xt{k}", name=f"xt{k}")
                  for k in range(8)]
            for b in range(2):
                for k in range(8):
                    nc.sync.dma_start(xt[k][:, b * L:(b + 1) * L],
                                      xT[k * 128:(k + 1) * 128, b * L:(b + 1) * L])
            win_t = p1.tile([128, 8 * 512], BF16, tag="win")
            nc.sync.dma_start(win_t[:], win[:])
            negs = [p1.tile([1, L], BF16, tag=f"negm{b}", name=f"negm{b}")
                    for b in range(2)]
            r_reps = [p1.tile([128, L], F32, tag="r_rep", name=f"r_rep{b}")
                      for b in range(2)]

            for b in range(2):
                # ---- LN1 stats for this half ----
                sum_sb = p1.tile([1, L], F32, tag="rows", bufs=3)
                sq_sb = p1.tile([1, L], F32, tag="rows", bufs=3)
                for ch in range(2):
                    sl = slice(b * L + ch * 512, b * L + (ch + 1) * 512)
                    dsl = slice(ch * 512, (ch + 1) * 512)
                    pss = ps_st.tile([1, 512], F32, tag="pstat", bufs=2)
                    for k in range(8):
                        nc.tensor.matmul(pss[:], ones_t[:], xt[k][:, sl],
                                         start=(k == 0), stop=(k == 7))
                    nc.vector.tensor_copy(sum_sb[:, dsl], pss[:])
                    psq = ps_st.tile([1, 512], F32, tag="pstat", bufs=2)
                    for k in range(8):
                        xq = p1sq.tile([128, 512], BF16, tag="xq")
                        nc.scalar.activation(xq[:], xt[k][:, sl], AF.Square)
                        nc.tensor.matmul(psq[:], ones_t[:], xq[:],
                                         start=(k == 0), stop=(k == 7))
                    nc.vector.tensor_copy(sq_sb[:, dsl], psq[:])
                m_neg = p1.tile([1, L], F32, tag="rows", bufs=3)
                nc.vector.tensor_scalar_mul(m_neg[:], sum_sb[:], -1.0 / E)
                nc.vector.tensor_copy(negs[b][:], m_neg[:])
                msq = p1.tile([1, L], F32, tag="rows", bufs=3)
                nc.vector.tensor_tensor(msq[:], m_neg[:], m_neg[:], OP.mult)
                var = p1.tile([1, L], F32, tag="rows", bufs=3)
                nc.vector.scalar_tensor_tensor(var[:], sq_sb[:], 1.0 / E,
                                               msq[:], OP.mult, OP.subtract)
                lnv = p1.tile([1, L], F32, tag="rows", bufs=3)
                nc.scalar.activation(lnv[:], var[:], AF.Ln, bias=eps_t[0:1, :])
                r_sb = p1.tile([1, L], F32, tag="rows", bufs=3)
                nc.scalar.activation(r_sb[:], lnv[:], AF.Exp, scale=-0.5)
                # broadcast rstd across partitions via a K=1 matmul
                for hh in range(2):
                    prb = ps_rb.tile([128, 512], F32, tag="prb")
                    nc.tensor.matmul(prb[:], ones_row_f[:],
                                     r_sb[:, hh * 512:(hh + 1) * 512],
                                     start=True, stop=True)
                    nc.vector.tensor_copy(r_reps[b][:, hh * 512:(hh + 1) * 512],
                                          prb[:])

                # ---- in_proj for this half ----
                for ch in range(2):
                    col = ch * 512
                    sl = slice(b * L + col, b * L + col + 512)
                    for mt in range(4):
                        ps = ps_in.tile([128, 512], F32, tag="ps", bufs=3)
                        for k in range(8):
                            nc.tensor.matmul(
                                ps[:],
                                win_t[:, k * 512 + mt * 128:k * 512 + (mt + 1) * 128],
                                xt[k][:, sl], start=(k == 0), stop=False)
                        nc.tensor.matmul(ps[:], sw_t[:, mt * 128:(mt + 1) * 128],
                                         negs[b][:, col:col + 512],
                                         start=False, stop=True)
                        if mt < 2:
                            dst = xmp[mt][:, b, 3 + col:3 + col + 512]
                        else:
                            dst = zt[mt - 2][:, b, col:col + 512]
                        if ln1b_nonzero:
                            tmp = p1sq.tile([128, 512], F32, tag="eptmp")
                            nc.vector.tensor_tensor(tmp[:], ps[:],
                                                    r_reps[b][:, col:col + 512],
                                                    OP.mult)
                            nc.scalar.activation(dst, tmp[:], AF.Identity,
                                                 bias=sbin_t[:, mt:mt + 1])
                        else:
                            nc.vector.tensor_tensor(dst, ps[:],
                                                    r_reps[b][:, col:col + 512],
                                                    OP.mult)

                # ---- conv + silu for this half ----
                for d in range(2):
                    acc0 = cvp.tile([128, L], BF16, tag="acc", bufs=3)
                    nc.vector.tensor_scalar_mul(acc0[:], xmp[d][:, b, 0:L],
                                                convw_t[:, d * KC:d * KC + 1])
                    for k in (1, 2, 3):
                        acc1 = cvp.tile([128, L], BF16, tag="acc", bufs=3)
                        nc.vector.scalar_tensor_tensor(
                            acc1[:], xmp[d][:, b, k:k + L],
                            convw_t[:, d * KC + k:d * KC + k + 1],
                            acc0[:], OP.mult, OP.add)
                        acc0 = acc1
                    nc.scalar.activation(xs[d][:, b, :], acc0[:], AF.Silu,
                                         bias=convb_t[:, d:d + 1])
                    nc.scalar.activation(zs[d][:, b, :], zt[d][:, b, :], AF.Silu)

                # ---- x_proj partial + AllReduce for this half ----
                xdblp = xpp.tile([96, L], F32, tag="xdblp")
                for ch in range(2):
                    col = ch * 512
                    psx = ps_xp.tile([96, 512], F32, tag="psx")
                    for k in range(2):
                        nc.tensor.matmul(psx[:], xpw_t[:, k * 96:(k + 1) * 96],
                                         xs[k][:, b, col:col + 512],
                                         start=(k == 0), stop=(k == 1))
                    nc.vector.tensor_copy(xdblp[:, col:col + 512], psx[:])
                nc.sync.dma_start(ar_ins[b][:], xdblp[:])
                nc.gpsimd.collective_compute("AllReduce", OP.add,
                                             ins=[ar_ins[b][:]],
                                             outs=[ar_outs[b][:]],
                                             replica_groups=RG)

            for b in range(2):
                dtr_b = dts.tile([64, L], BF16, tag="dtrb")
                nc.gpsimd.dma_start(dtr_b[:], ar_outs[b][0:64, :])
                nc.gpsimd.dma_start(bc_bfs[b][:], ar_outs[b][64:96, :])
                for mt in range(2):
                    dte_t = dts.tile([128, L], BF16, tag="dte")
                    for ch in range(2):
                        col = ch * 512
                        psd = ps_in.tile([128, 512], F32, tag="ps", bufs=3)
                        nc.tensor.matmul(psd[:],
                                         dtw_t[:, mt * 128:(mt + 1) * 128],
                                         dtr_b[:, col:col + 512],
                                         start=True, stop=True)
                        nc.scalar.activation(dte_t[:, col:col + 512],
                                             psd[:], AF.Exp,
                                             bias=dtb_t[:, mt:mt + 1])
                    nc.scalar.activation(dt_t[mt][:, b, :], dte_t[:],
                                         AF.Ln, bias=1.0)
                    dtxb = dts.tile([128, L], BF16, tag="dtxb")
                    nc.vector.tensor_copy(dtxb[:], dt_t[mt][:, b, :])
                    nc.vector.tensor_tensor(dtx[mt][:, b, :], dtxb[:],
                                            xs[mt][:, b, :], OP.mult)

        # ===== Phase 5: scan (d-outer) + per-d AllToAll + split-K out_proj ====
        _xzstk.close()
        opp = _stk.enter_context(tc.tile_pool(name="op", bufs=1, side="right"))
        r1 = [opp.tile([128, TOKC], F32, tag=f"r1_{m}", name=f"r1_{m}")
              for m in range(8)]
        r1b = [opp.tile([128, TOKC], BF16, tag=f"r1b{m}", name=f"r1b{m}")
               for m in range(8)]
        G = 4
        NG = NST // G
        with tc.tile_pool(name="ps_po", bufs=1, space="PSUM") as ps_po, \
             tc.tile_pool(name="wop", bufs=1) as wop:
            po = [ps_po.tile([128, 512], F32, tag=f"po{i}", name=f"po{i}")
                  for i in range(4)]
            yf = [[wop.tile([128, TOKC], BF16, tag=f"yf{d}_{i}",
                            name=f"yf{d}_{i}") for i in range(NC)]
                  for d in range(2)]
            _scanstk = ExitStack()
            pa = _scanstk.enter_context(tc.tile_pool(name="scan_a", bufs=2))
            pbh = _scanstk.enter_context(tc.tile_pool(name="scan_bh", bufs=3))
            pr = _scanstk.enter_context(tc.tile_pool(name="scan_r", bufs=3))
            py = _scanstk.enter_context(tc.tile_pool(name="scan_y", bufs=1))
            ps_y = _scanstk.enter_context(
                tc.tile_pool(name="ps_y", bufs=2, space="PSUM"))
            for d in range(2):
                for b in range(2):
                    psy = ps_y.tile([128, L], F32, tag="psy")
                    for g in range(NG):
                        a_t = pa.tile([128, G, L], BF16, tag="a")
                        for j in range(G):
                            n = g * G + j
                            nc.scalar.activation(
                                a_t[:, j, :], dt_t[d][:, b, :], AF.Exp,
                                scale=asc_t[:, d * NST + n:d * NST + n + 1])
                        nc.vector.memset(a_t[:, :, 0:1], 0.0)
                        brep = pr.tile([128, G, L], BF16, tag="bcr")
                        nc.sync.dma_start(brep[:],
                                          _rep0(bc_bfs[b][g * G:(g + 1) * G, :]))
                        bx = pbh.tile([128, G, L], BF16, tag="bxhc")
                        dslice = dtx[d][:, b, :]
                        dxb = bass.AP(dslice.tensor, dslice.offset,
                                      [list(dslice.ap[0]), [0, G], [1, L]])
                        nc.vector.tensor_tensor(bx[:], dxb, brep[:], OP.mult)
                        h_t = pbh.tile([128, G, L], BF16, tag="bxhc")
                        nc.vector.tensor_tensor_scan(
                            h_t[:].rearrange("p a b -> p (a b)"),
                            a_t[:].rearrange("p a b -> p (a b)"),
                            bx[:].rearrange("p a b -> p (a b)"),
                            0.0, OP.mult, OP.add)
                        crep = pr.tile([128, G, L], BF16, tag="bcr")
                        nc.sync.dma_start(crep[:],
                                          _rep0(bc_bfs[b][16 + g * G:16 + (g + 1) * G, :]))
                        hc = pbh.tile([128, G, L], BF16, tag="bxhc")
                        nc.vector.tensor_tensor(hc[:], h_t[:], crep[:], OP.mult)
                        for j in range(G):
                            for hh in range(2):
                                nc.tensor.matmul(
                                    psy[:, hh * 512:(hh + 1) * 512], ident_t[:],
                                    hc[:, j, hh * 512:(hh + 1) * 512],
                                    start=(g == 0 and j == 0),
                                    stop=(g == NG - 1 and j == G - 1))
                    y2 = py.tile([128, L], BF16, tag="y2")
                    nc.vector.scalar_tensor_tensor(y2[:], xs[d][:, b, :],
                                                   dvec_t[:, d:d + 1], psy[:],
                                                   OP.mult, OP.add)
                    nc.vector.tensor_tensor(y3[d][:, b, :], y2[:],
                                            zs[d][:, b, :], OP.mult)
                # ---- A2A for this d-half + out_proj half-K accumulation ----
                for c in range(NC):
                    nc.sync.dma_start(
                        a2a_ins[d][c].rearrange("(p q) -> p q", p=128),
                        y3[d][:, c // 4, (c % 4) * TOKC:(c % 4 + 1) * TOKC])
                nc.gpsimd.collective_compute("AllToAll", OP.bypass,
                                             ins=[a2a_ins[d][:]],
                                             outs=[a2a_outs[d][:]],
                                             replica_groups=RG)
                wo_h = wop.tile([128, 8 * 1024], BF16, tag="wo", name=f"wo_h{d}")
                nc.sync.dma_start(wo_h[:], wo[:, d * 8192:(d + 1) * 8192])
                for i in range(NC):
                    nc.sync.dma_start(
                        yf[d][i][:],
                        a2a_outs[d][i].rearrange("(p q) -> p q", p=128))
                for mt in range(8):
                    out_ap = po[mt // 2][:, (mt % 2) * 256:(mt % 2) * 256 + 256]
                    for i in range(NC):
                        nc.tensor.matmul(
                            out_ap,
                            wo_h[:, i * 1024 + mt * 128:i * 1024 + (mt + 1) * 128],
                            yf[d][i][:],
                            start=(d == 0 and i == 0 and mt % 2 == 0),
                            stop=(d == 1 and i == NC - 1),
                            skip_group_check=True)
            _scanstk.close()
            for mt in range(8):
                out_ap = po[mt // 2][:, (mt % 2) * 256:(mt % 2) * 256 + 256]
                nc.vector.tensor_tensor(r1[mt][:], out_ap, xres_t[mt][:], OP.add)
                nc.vector.tensor_copy(r1b[mt][:], r1[mt][:])
        _mstk.close()
        _mlppre = ExitStack()
        mlpp = _mlppre.enter_context(tc.tile_pool(name="mlp", bufs=1))
        _wfcstk = ExitStack()
        wfcp = _wfcstk.enter_context(tc.tile_pool(name="wfcp", bufs=1, side="right"))
        wfc_t = wfcp.tile([128, 8 * HID], BF16, tag="wfc")
        nc.sync.dma_start(wfc_t[:], wfc[:])

        # ================= Phase 6: LN2 + MLP (token-sharded) =================
        with ExitStack() as _s2:
            ps_s2 = _s2.enter_context(
                tc.tile_pool(name="ps_s2", bufs=1, space="PSUM"))
            sum2 = opp.tile([1, TOKC], F32, tag="sum2")
            sq2 = opp.tile([1, TOKC], F32, tag="sq2")
            ps2a = ps_s2.tile([1, TOKC], F32, tag="ps2a")
            for k in range(8):
                nc.tensor.matmul(ps2a[:], ones_t[:], r1b[k][:],
                                 start=(k == 0), stop=(k == 7))
            nc.vector.tensor_copy(sum2[:], ps2a[:])
            ps2b = ps_s2.tile([1, TOKC], F32, tag="ps2b")
            for k in range(8):
                q2 = opp.tile([128, TOKC], BF16, tag="q2")
                nc.vector.tensor_tensor(q2[:], r1b[k][:], r1b[k][:], OP.mult)
                nc.tensor.matmul(ps2b[:], ones_t[:], q2[:],
                                 start=(k == 0), stop=(k == 7))
            nc.vector.tensor_copy(sq2[:], ps2b[:])
            m2n = opp.tile([1, TOKC], F32, tag="m2n")
            nc.vector.tensor_scalar_mul(m2n[:], sum2[:], -1.0 / E)
            msq2 = opp.tile([1, TOKC], F32, tag="msq2")
            nc.vector.tensor_tensor(msq2[:], m2n[:], m2n[:], OP.mult)
            var2 = opp.tile([1, TOKC], F32, tag="var2")
            nc.vector.scalar_tensor_tensor(var2[:], sq2[:], 1.0 / E, msq2[:],
                                           OP.mult, OP.subtract)
            lnv2 = opp.tile([1, TOKC], F32, tag="lnv2")
            nc.scalar.activation(lnv2[:], var2[:], AF.Ln, bias=eps_t[0:1, :])
            r2_sb = opp.tile([1, TOKC], F32, tag="r2_sb")
            nc.scalar.activation(r2_sb[:], lnv2[:], AF.Exp, scale=-0.5)
            m2b = opp.tile([1, TOKC], BF16, tag="m2b")
            nc.vector.tensor_copy(m2b[:], m2n[:])
            pr2 = ps_s2.tile([128, TOKC], F32, tag="pr2")
            nc.tensor.matmul(pr2[:], ones_row_f[:],
                             r2_sb[:], start=True, stop=True)
            r2_rep = opp.tile([128, TOKC], F32, tag="r2rep")
            nc.vector.tensor_copy(r2_rep[:], pr2[:])

        with tc.tile_pool(name="fcep", bufs=3) as fcep:
            h1 = [mlpp.tile([128, TOKC], BF16, tag=f"h1_{m}", name=f"h1_{m}")
                  for m in range(32)]
            wpj_t = mlpp.tile([128, 32 * E], BF16, tag="wpj")
            nc.sync.dma_start(wpj_t[:], wpj[:])
            with tc.tile_pool(name="ps_f", bufs=6, space="PSUM") as ps_f:
                for mt in range(32):
                    psf = ps_f.tile([128, TOKC], F32, tag="psf")
                    for k in range(8):
                        nc.tensor.matmul(
                            psf[:],
                            wfc_t[:, k * HID + mt * 128:k * HID + (mt + 1) * 128],
                            r1b[k][:], start=(k == 0), stop=False)
                    nc.tensor.matmul(psf[:], swfc_t[:, mt * 128:(mt + 1) * 128],
                                     m2b[:], start=False, stop=True)
                    tmp = fcep.tile([128, TOKC], F32, tag="fctmp")
                    nc.vector.tensor_tensor(tmp[:], psf[:], r2_rep[:], OP.mult)
                    nc.scalar.activation(h1[mt][:], tmp[:], AF.Gelu,
                                         bias=sbfc_t[:, mt:mt + 1])
            _wfcstk.close()
            with tc.tile_pool(name="ps_p", bufs=1, space="PSUM") as ps_p, \
                 tc.tile_pool(name="pjep", bufs=2) as pjep:
                psps = [ps_p.tile([128, TOKC], F32, tag=f"psp{m}",
                                  name=f"psp{m}") for m in range(8)]
                for k in range(32):
                    for mt in range(8):
                        nc.tensor.matmul(psps[mt][:],
                                         wpj_t[:, k * E + mt * 128:k * E + (mt + 1) * 128],
                                         h1[k][:], start=(k == 0),
                                         stop=(k == 31))
                for mt in range(8):
                    ot = pjep.tile([128, TOKC], F32, tag="ot")
                    nc.vector.scalar_tensor_tensor(ot[:], psps[mt][:],
                                                   pjb_t[:, mt:mt + 1],
                                                   r1[mt][:], OP.add, OP.add)
                    nc.sync.dma_start(outT[mt * 128:(mt + 1) * 128, :], ot[:])
        _mlppre.close()

    nc.compile()
    _BUILD_CACHE[key] = nc
    return nc


def _prep_inputs(inputs):
    """Host-side sharding/packing. Returns list of per-core input dicts."""
    f32 = np.float32
    x = np.asarray(inputs["x"], f32)
    ln1_w = np.asarray(inputs["ln1_w"], f32)
    ln1_b = np.asarray(inputs["ln1_b"], f32)
    in_proj_w = np.asarray(inputs["in_proj_w"], f32)
    conv_w = np.asarray(inputs["conv_w"], f32)
    conv_b = np.asarray(inputs["conv_b"], f32)
    x_proj_w = np.asarray(inputs["x_proj_w"], f32)
    dt_proj_w = np.asarray(inputs["dt_proj_w"], f32)
    dt_proj_b = np.asarray(inputs["dt_proj_b"], f32)
    A_log = np.asarray(inputs["A_log"], f32)
    D = np.asarray(inputs["D"], f32)
    out_proj_w = np.asarray(inputs["out_proj_w"], f32)
    ln2_w = np.asarray(inputs["ln2_w"], f32)
    ln2_b = np.asarray(inputs["ln2_b"], f32)
    fc_w = np.asarray(inputs["fc_w"], f32)
    fc_b = np.asarray(inputs["fc_b"], f32)
    proj_w = np.asarray(inputs["proj_w"], f32)
    proj_b = np.asarray(inputs["proj_b"], f32)

    xT_f = np.ascontiguousarray(x.reshape(TOK, E).T)          # [E, TOK]
    xT_b = xT_f.astype(bf)

    def pack_lhsT(lhsT):
        K, M = lhsT.shape
        nk = K // 128
        return np.ascontiguousarray(
            lhsT.reshape(nk, 128, M).transpose(1, 0, 2).reshape(128, nk * M)
        ).astype(bf)

    Wp = in_proj_w * ln1_w[None, :]
    sb_full = in_proj_w @ ln1_b
    ln1b_nonzero = bool(np.any(sb_full != 0.0))

    Wfc = fc_w * ln2_w[None, :]
    swfc_full = Wfc.sum(1)
    sbfc_full = fc_w @ ln2_b + fc_b
    wfc_pack = pack_lhsT(np.ascontiguousarray(Wfc.T))
    wpj_pack = pack_lhsT(np.ascontiguousarray(proj_w.T))
    wo_pack0 = pack_lhsT(np.ascontiguousarray(out_proj_w.T))  # [128, 16*1024]
    blocks = [wo_pack0[:, k * 1024:(k + 1) * 1024] for k in range(16)]
    wo_pack = np.ascontiguousarray(
        np.concatenate([b for k, b in enumerate(blocks) if k % 2 == 0] +
                       [b for k, b in enumerate(blocks) if k % 2 == 1], axis=1))
    pjb_pack = np.ascontiguousarray(proj_b.reshape(8, 128).T).astype(f32)
    sbfc_pack = np.ascontiguousarray(sbfc_full.reshape(32, 128).T).astype(f32)
    swfc_row = swfc_full[None, :].astype(bf)

    A = -np.exp(A_log)

    per_core = []
    for c in range(NC):
        dsl = slice(c * DL, (c + 1) * DL)
        rows = np.concatenate([Wp[dsl], Wp[DIN + c * DL:DIN + (c + 1) * DL]])
        win_pack = pack_lhsT(np.ascontiguousarray(rows.T))
        sw_row = rows.sum(1)[None, :].astype(bf)
        sb_rows = np.concatenate([sb_full[dsl],
                                  sb_full[DIN + c * DL:DIN + (c + 1) * DL]])
        sb_pack = np.ascontiguousarray(sb_rows.reshape(4, 128).T).astype(f32)

        cw = conv_w[dsl, 0, :]
        convw_pack = np.ascontiguousarray(
            cw.reshape(2, 128, KC).transpose(1, 0, 2).reshape(128, 2 * KC)
        ).astype(f32)
        convb_pack = np.ascontiguousarray(
            conv_b[dsl].reshape(2, 128).T).astype(f32)

        xpw_pack = pack_lhsT(np.ascontiguousarray(x_proj_w[:, dsl].T))
        dtw_slice = np.ascontiguousarray(dt_proj_w[dsl].T).astype(bf)
        dtb_pack = np.ascontiguousarray(
            dt_proj_b[dsl].reshape(2, 128).T).astype(f32)
        asc_pack = np.ascontiguousarray(
            A[dsl].reshape(2, 128, NST).transpose(1, 0, 2).reshape(128, 2 * NST)
        ).astype(f32)
        dvec_pack = np.ascontiguousarray(D[dsl].reshape(2, 128).T).astype(f32)

        xres_slice = np.ascontiguousarray(xT_f[:, c * TOKC:(c + 1) * TOKC])

        per_core.append({
            "xT": xT_b, "win": win_pack, "sw_in": sw_row, "sb_in": sb_pack,
            "convw": convw_pack, "convb": convb_pack, "xpw": xpw_pack,
            "dtw": dtw_slice, "dtb": dtb_pack, "a_sc": asc_pack,
            "dvec": dvec_pack, "wo": wo_pack, "xres": xres_slice,
            "wfc": wfc_pack, "swfc": swfc_row, "sbfc": sbfc_pack,
            "wpj": wpj_pack, "pjb": pjb_pack,
            "ones128": np.ones((128, 1), bf),
            "ident": np.eye(128, dtype=bf),
        })
    return per_core, ln1b_nonzero


def kernel(**inputs):
    per_core, ln1b_nonzero = _prep_inputs(inputs)
    nc = _build(ln1b_nonzero)
    res = run_bass_kernel_spmd(
        nc, per_core, core_ids=list(range(NC)),
        trace=bool(int(os.environ.get("BASSK_TRACE", "0"))),
    )
    kernel.last_results = res
    out_T = np.concatenate([res.results[c]["outT"] for c in range(NC)], axis=1)
    return np.ascontiguousarray(out_T.T).reshape(B, L, E).astype(np.float32)
